# revision 36
# baseline (speedup 1.0000x reference)
"""Trainium2 Bass kernel for the ButterflyMlp problem.

Computes log_softmax(L3(relu(L2(relu(L1(x)))))) where each Li is a masked
linear layer (butterfly sparsity: global column stripes + a diagonal band),
batch 65536, data-parallel over 8 NeuronCores (8192 rows/core).

Strategy (per core, feature-major throughout):
  - L1 exploits butterfly structure: stripe columns S (204, dense for every
    output row) are one shared K-axis; each 112-row output block adds a
    narrow band residual (<=92 cols). Both run as fp8e4 DoubleRow matmuls
    (two 128-row K-planes per pass, 0.5 cycles/col): 2 matmuls per block.
  - L1 bias is folded into the stripe matmul via an appended ones-row in the
    packed x (weight row = b1), so PSUM evictions are pure ReLU+cast ops,
    split round-robin across ScalarE/DVE/Pool.
  - L2 (K=784) runs as 3 fp8 DoubleRow pair matmuls over paired y1 tiles
    plus one fp16 tail matmul; bias b2 via ACT on the y2 eviction.
  - L3 (K=128, C=10) is fp16 with C padded to 32; four consecutive batch
    chunks write one PSUM bank at partition offsets 0/32/64/96 (PE array
    tiling), so exp/ln/subtract of log_softmax batch 4 chunks per pass and
    the logsumexp reduction is a single ones-blockdiag matmul.
  - x is pre-gathered and fp8-packed on host; per-slab DMAs go through
    SWDGE (nc.gpsimd) with 4-28KB contiguous runs per partition; all xs
    slabs are prefetched ahead of the larger xb slabs. Outputs stream back
    on the sync HWDGE ring.
"""
import sys
sys.path.insert(0, "/opt/trn_rl_repo")
import numpy as np
import ml_dtypes

import concourse.bass as bass
import concourse.bacc as bacc
import concourse.mybir as mybir
import concourse.tile as tile
from concourse import bass_utils

F32 = mybir.dt.float32
F16 = mybir.dt.float16
F8 = mybir.dt.float8e4
NP8 = ml_dtypes.float8_e4m3
AF = mybir.ActivationFunctionType
ALU = mybir.AluOpType
DR = mybir.MatmulPerfMode.DoubleRow

# Keep every ACT function this kernel uses (Relu/Exp/Ln + implicit Copy /
# Identity) inside one activation-table set so the greedy per-function set
# chooser emits a single table load instead of reloading per chunk.
_PIN_SET = "natural_log_exp_and_others"
_orig_gat = bacc.get_activation_tables


def _pinned_gat(arch):
    tabs = _orig_gat(arch)
    need = {AF.Relu, AF.Identity, AF.Exp, AF.Ln, AF.Copy}
    if _PIN_SET in tabs and need <= tabs[_PIN_SET]:
        for name in tabs:
            if name != _PIN_SET:
                tabs[name] = tabs[name] - need
    return tabs


bacc.get_activation_tables = _pinned_gat

N_CORES = 8
NB = 512          # batch columns per matmul (one fp32 PSUM bank)
SC = 2048         # batch columns per DMA slab
OT = 112          # L1 output block width (784/7)
GROUP = 3         # batch chunks per L3/log-softmax group (3*32 = 96 rows;
                  # AP base partitions are limited to 0/32/64)
CP = 32           # padded class count (PE tile col granularity)


def _decompose_mask1(mask1):
    D_out, D_in = mask1.shape
    S = np.where(mask1.all(axis=0))[0]
    n_blk = (D_out + OT - 1) // OT
    stripe_set = np.zeros(D_in, dtype=bool)
    stripe_set[S] = True
    R_list = []
    for j in range(n_blk):
        blk = mask1[j * OT:(j + 1) * OT]
        R_list.append(np.where(blk.any(axis=0) & ~stripe_set)[0])
    return S, R_list


def _build_program(meta):
    PS, PB = meta["PS"], meta["PB"]
    Bc = meta["Bc"]
    D1, H, C = meta["D1"], meta["H"], meta["C"]
    n_blk = D1 // OT
    n_pair = n_blk // 2                 # L2 DoubleRow pairs (tail is fp16)
    n_sup = Bc // SC
    n_chunk = Bc // NB

    nc = bacc.Bacc("TRN2", target_bir_lowering=False, debug=False,
                   enable_asserts=False, num_devices=N_CORES)

    xs_d = nc.dram_tensor("xs", [PS, n_sup * 2 * SC], F8,
                          kind="ExternalInput").ap()
    xb_d = nc.dram_tensor("xb", [PB, n_sup * n_blk * SC], F8,
                          kind="ExternalInput").ap()
    ws_d = nc.dram_tensor("ws", [PS, n_blk * 2 * OT], F8,
                          kind="ExternalInput").ap()
    wb_d = nc.dram_tensor("wb", [PB, n_blk * OT], F8,
                          kind="ExternalInput").ap()
    w2_d = nc.dram_tensor("w2", [OT, n_blk * H], F8,
                          kind="ExternalInput").ap()
    w3_d = nc.dram_tensor("w3", [H, CP], F16, kind="ExternalInput").ap()
    ones_d = nc.dram_tensor("ones", [GROUP * CP, GROUP * CP], F16,
                            kind="ExternalInput").ap()
    b2_d = nc.dram_tensor("b2", [H, 1], F32, kind="ExternalInput").ap()
    b3t_d = nc.dram_tensor("b3t", [GROUP * CP, 1], F32,
                           kind="ExternalInput").ap()
    out_d = nc.dram_tensor("out", [C, Bc], F32, kind="ExternalOutput").ap()

    with tile.TileContext(nc) as tc:
        with tc.tile_pool(name="wp", bufs=1) as wp, \
             tc.tile_pool(name="xsp", bufs=n_sup) as xsp, \
             tc.tile_pool(name="xbp", bufs=2) as xbp, \
             tc.tile_pool(name="y1p", bufs=15) as y1pp, \
             tc.tile_pool(name="y2p", bufs=4) as y2p, \
             tc.tile_pool(name="exp", bufs=2) as exp_p, \
             tc.tile_pool(name="y3p", bufs=2) as y3p, \
             tc.tile_pool(name="lsp", bufs=2) as lsp, \
             tc.tile_pool(name="op", bufs=2) as op, \
             tc.tile_pool(name="ps1", bufs=5, space="PSUM") as ps1, \
             tc.tile_pool(name="ps2", bufs=1, space="PSUM") as ps2, \
             tc.tile_pool(name="ps34", bufs=2, space="PSUM") as ps34:

            # ---- resident weights (SWDGE, ahead of the x slabs) ----
            ws_sb = wp.tile([PS, n_blk, 2, OT], F8)
            nc.gpsimd.dma_start(ws_sb[:], ws_d[:])
            wb_sb = wp.tile([PB, n_blk, OT], F8)
            nc.gpsimd.dma_start(wb_sb[:], wb_d[:])
            w2_sb = wp.tile([OT, n_blk, H], F8)
            nc.gpsimd.dma_start(w2_sb[:], w2_d[:])
            w3_sb = wp.tile([H, CP], F16)
            nc.gpsimd.dma_start(w3_sb[:], w3_d[:])
            ones_sb = wp.tile([GROUP * CP, GROUP * CP], F16)
            nc.gpsimd.dma_start(ones_sb[:], ones_d[:])
            b2_sb = wp.tile([H, 1], F32)
            nc.sync.dma_start(b2_sb[:], b2_d[:])
            b3t_sb = wp.tile([GROUP * CP, 1], F32)
            nc.sync.dma_start(b3t_sb[:], b3t_d[:])

            # ---- x slab loads: all xs first (small), then xb per slab.
            # Chunk-major layout: every per-chunk moving slice is fully
            # contiguous (the PE's fast path needs packed moving APs).
            n_half = SC // NB
            xs_tiles = []
            for s in range(n_sup):
                t = xsp.tile([PS, n_half, 2, NB], F8, name=f"xs{s}", tag="xs")
                nc.gpsimd.dma_start(t[:], xs_d[:, s * 2 * SC:(s + 1) * 2 * SC])
                xs_tiles.append(t)
            xb_tiles = []
            for s in range(n_sup):
                t = xbp.tile([PB, n_half, n_blk, NB], F8,
                             name=f"xb{s}", tag="xb")
                nc.gpsimd.dma_start(
                    t[:], xb_d[:, s * n_blk * SC:(s + 1) * n_blk * SC])
                xb_tiles.append(t)

            # Per-chunk state threaded through the software pipeline.
            y1_tiles = {}    # c -> [7 y1 tiles]
            p2_tiles = {}    # c -> L2 PSUM tile
            y2_tiles = {}    # c -> y2 SBUF tile
            p3_tiles = {}    # g -> L3 group PSUM tile
            ex_tiles = {}    # g -> exp SBUF tile
            y3_tiles = {}    # g -> logits+bias SBUF tile
            pending_tails = []  # (g, body_ready) awaiting softmax tail
            gsize = lambda g: min(GROUP, n_chunk - g * GROUP)

            def emit_l2_all(c):
                # The whole L2 for chunk c, deferred one body: every y1
                # eviction is long done, so the PE never waits here. The y2
                # eviction is NOT emitted here: it would head the epilogue
                # queue and block this body's L1 evictions behind the L2
                # chain. The body emits it after its second L1 eviction.
                p2_tiles[c] = ps2.tile([H, NB], F32, tag="l2", name="p2")
                for k in range(n_blk):
                    nc.tensor.matmul(p2_tiles[c][:], w2_sb[:, k, :],
                                     y1_tiles[c][k][:], start=(k == 0),
                                     stop=(k == n_blk - 1))
                del y1_tiles[c]

            def emit_y2_evict(c):
                y2 = y2p.tile([H, NB], F16, tag="y2")
                if c % 2 == 0:
                    nc.scalar.activation(y2[:], p2_tiles[c][:], AF.Relu,
                                         bias=b2_sb[:, 0:1])
                else:
                    nc.vector.tensor_scalar(y2[:], p2_tiles[c][:],
                                            b2_sb[:, 0:1], 0.0,
                                            op0=ALU.add, op1=ALU.max)
                y2_tiles[c] = y2
                del p2_tiles[c]

            def emit_l3_group(g, body):
                # All of the group's col-32 L3 matmuls back to back: the PE
                # column-tile reconfig (128 -> 32 -> 128) is paid once per
                # group instead of once per chunk.
                gs = gsize(g)
                gp = gs * CP
                p3 = ps34.tile([GROUP * CP, NB], F32, tag="l34", name="p3")
                for m in range(gs):
                    c = g * GROUP + m
                    nc.tensor.matmul(p3[m * CP:(m + 1) * CP, :],
                                     w3_sb[:], y2_tiles[c][:],
                                     start=True, stop=True)
                    del y2_tiles[c]
                p3_tiles[g] = p3

            def emit_exp(g, body):
                # Exp + bias-add for a finished group — placed at body end so
                # the epilogue engines drain this body's evictions first.
                gs = gsize(g)
                gp = gs * CP
                p3 = p3_tiles.pop(g)
                ex = exp_p.tile([GROUP * CP, NB], F16, tag="ex")
                nc.scalar.activation(ex[:gp, :], p3[:gp, :],
                                     AF.Exp, bias=b3t_sb[:gp, 0:1])
                y3 = y3p.tile([GROUP * CP, NB], F32, tag="y3")
                nc.vector.tensor_scalar(y3[:gp, :], p3[:gp, :],
                                        b3t_sb[:gp, 0:1], None, op0=ALU.add)
                ex_tiles[g] = ex
                y3_tiles[g] = y3
                pending_tails.append((g, body))

            def emit_softmax_tail(g):
                gs = gsize(g)
                gp = gs * CP
                psl = ps34.tile([GROUP * CP, NB], F32, tag="l34", name="psl")
                nc.tensor.matmul(psl[:gp, :], ones_sb[:gp, :gp],
                                 ex_tiles[g][:gp, :], start=True, stop=True)
                ls = lsp.tile([GROUP * CP, NB], F32, tag="ls")
                nc.scalar.activation(ls[:gp, :], psl[:gp, :], AF.Ln)
                o = op.tile([GROUP * CP, NB], F32, tag="o")
                nc.gpsimd.tensor_tensor(o[:gp, :], y3_tiles[g][:gp, :],
                                        ls[:gp, :], op=ALU.subtract)
                for m in range(gs):
                    cc = g * GROUP + m
                    nc.sync.dma_start(out_d[:, cc * NB:(cc + 1) * NB],
                                      o[m * CP:m * CP + C, :])
                del ex_tiles[g], y3_tiles[g]

            def flush_tails(body):
                while pending_tails and pending_tails[0][1] < body:
                    emit_softmax_tail(pending_tails.pop(0)[0])

            def l1_mms(dst_psum, j, xs_t, xb_t, h):
                nc.tensor.matmul(dst_psum, ws_sb[:, j, 0, :],
                                 xs_t[:, h, 0, :],
                                 start=True, stop=False)
                nc.tensor.matmul(dst_psum, ws_sb[:, j, 1, :],
                                 xs_t[:, h, 1, :],
                                 start=False, stop=False)
                nc.tensor.matmul(dst_psum, wb_sb[:, j, :],
                                 xb_t[:, h, j, :],
                                 start=False, stop=True)

            for c in range(n_chunk):
                s, h = c // (SC // NB), c % (SC // NB)
                xs_t, xb_t = xs_tiles[s], xb_tiles[s]

                # Deferred work from earlier chunks heads the PE stream:
                # their data deps resolved most of a body ago, so no stalls.
                if c >= 1:
                    emit_l2_all(c - 1)
                if c >= 2 and ((c - 2) % GROUP == GROUP - 1
                               or c - 2 == n_chunk - 1):
                    emit_l3_group((c - 2) // GROUP, c)

                # Seven L1 blocks, each three fp8 matmuls into one PSUM
                # bank, evicted by a pure ReLU+cast alternating ACT/DVE.
                y1s = []
                for j in range(n_blk):
                    p1 = ps1.tile([OT, NB], F32, tag="l1", name="p1")
                    l1_mms(p1[:], j, xs_t, xb_t, h)
                    y1 = y1pp.tile([OT, NB], F8, tag="y1")
                    if j % 2 == 0:
                        nc.scalar.activation(y1[:], p1[:], AF.Relu)
                    else:
                        nc.vector.tensor_scalar(y1[:], p1[:], 0.0, None,
                                                op0=ALU.max)
                    y1s.append(y1)
                    if j == 2 and c >= 1:
                        emit_y2_evict(c - 1)
                y1_tiles[c] = y1s

                # Group tail work last: the epilogue engines finish this
                # body's evictions before touching exp/ln/subtract.
                if c >= 2 and ((c - 2) % GROUP == GROUP - 1
                               or c - 2 == n_chunk - 1):
                    emit_exp((c - 2) // GROUP, c)
                flush_tails(c)

            # ---- drain the pipeline ----
            emit_l2_all(n_chunk - 1)
            emit_y2_evict(n_chunk - 1)
            for cc in (n_chunk - 2, n_chunk - 1):
                if cc % GROUP == GROUP - 1 or cc == n_chunk - 1:
                    g = cc // GROUP
                    emit_l3_group(g, n_chunk)
                    emit_exp(g, n_chunk)
            for g, _ in list(pending_tails):
                emit_softmax_tail(g)
            pending_tails.clear()

    nc.compile()
    return nc


_CACHE = {}


def _prepare(x, W1, b1, W2, b2, W3, b3, mask1, mask2, mask3):
    B, D1 = x.shape
    H = W2.shape[0]
    C = W3.shape[0]
    assert B % N_CORES == 0
    Bc = B // N_CORES
    n_blk = D1 // OT
    n_pair = n_blk // 2
    n_sup = Bc // SC

    S, R_list = _decompose_mask1(np.asarray(mask1))
    nS = len(S)
    PS = (nS + 2 + 1) // 2              # stripe K-planes incl ones+zero rows
    maxR = max(len(r) for r in R_list)
    # >=65 partitions keeps the PE in its 128-row tile config: mixing 64-row
    # and 128-row matmuls in one stream costs a reconfig bubble per matmul.
    PB = max(maxR, 65)

    Wm1 = (np.asarray(W1) * np.asarray(mask1)).astype(np.float32)
    Wm2 = (np.asarray(W2) * np.asarray(mask2)).astype(np.float32)
    Wm3 = (np.asarray(W3) * np.asarray(mask3)).astype(np.float32)
    b1 = np.asarray(b1, np.float32)
    b2 = np.asarray(b2, np.float32)
    b3 = np.asarray(b3, np.float32)

    c8 = lambda a: np.asarray(a, dtype=NP8)
    c16 = lambda a: np.asarray(a, dtype=np.float16)

    # ---- stripe pack: K order = S cols, then ones row, then zero pad ----
    xT = np.asarray(x, np.float32).T                     # [D1, B]
    SP2 = 2 * PS
    xs_src = np.zeros((SP2, B), np.float32)
    xs_src[:nS] = xT[S]
    xs_src[nS] = 1.0
    # [NC, PS, n_sup, n_half, 2, NB] — chunk-major for contiguous slices
    n_half = SC // NB
    xs8 = (c8(xs_src).reshape(2, PS, N_CORES, n_sup, n_half, NB)
           .transpose(2, 1, 3, 4, 0, 5))
    xs8 = np.ascontiguousarray(xs8.reshape(N_CORES, PS, n_sup * 2 * SC))

    Ws_full = np.zeros((D1, SP2), np.float32)
    Ws_full[:, :nS] = Wm1[:, S]
    Ws_full[:, nS] = b1
    ws8 = np.ascontiguousarray(
        c8(Ws_full).reshape(n_blk, OT, 2, PS).transpose(3, 0, 2, 1)
        .reshape(PS, n_blk * 2 * OT))

    # ---- band pack (flat K, fp8 single matmuls) ----
    xb_src = np.zeros((n_blk, PB, B), np.float32)
    wb_src = np.zeros((n_blk, OT, PB), np.float32)
    for j, R in enumerate(R_list):
        xb_src[j, :len(R)] = xT[R]
        wb_src[j, :, :len(R)] = Wm1[j * OT:(j + 1) * OT, R]
    # [NC, PB, n_sup, n_half, n_blk, NB]
    xb8 = (c8(xb_src).reshape(n_blk, PB, N_CORES, n_sup, n_half, NB)
           .transpose(2, 1, 3, 4, 0, 5))
    xb8 = np.ascontiguousarray(
        xb8.reshape(N_CORES, PB, n_sup * n_blk * SC))
    wb8 = np.ascontiguousarray(
        c8(wb_src).transpose(2, 0, 1).reshape(PB, n_blk * OT))

    # ---- L2 pack: seven fp8 single matmuls ----
    t2 = Wm2.T.reshape(n_blk, OT, H)                     # [j, p, h]
    w2a8 = np.ascontiguousarray(
        c8(t2).transpose(1, 0, 2).reshape(OT, n_blk * H))

    # ---- L3 pack: classes padded to CP, 4 chunks per PE-tile group ----
    w3p = np.zeros((H, CP), np.float32)
    w3p[:, :C] = Wm3.T
    w3p16 = c16(w3p)
    GC = GROUP * CP
    ones = np.zeros((GC, GC), np.float32)
    for g in range(GROUP):
        ones[g * CP:g * CP + C, g * CP:g * CP + C] = 1.0
    ones16 = c16(ones)
    b3t = np.zeros((GC, 1), np.float32)
    for g in range(GROUP):
        b3t[g * CP:g * CP + C, 0] = b3
    b2p = b2.reshape(H, 1)

    meta = dict(PS=PS, PB=PB, Bc=Bc, D1=D1, H=H, C=C)
    key = (B, D1, H, C, nS, PB)
    if key not in _CACHE:
        _CACHE[key] = _build_program(meta)
    nc = _CACHE[key]

    in_maps = []
    for c in range(N_CORES):
        in_maps.append({
            "xs": xs8[c], "xb": xb8[c],
            "ws": ws8, "wb": wb8, "w2": w2a8,
            "w3": w3p16, "ones": ones16, "b2": b2p, "b3t": b3t,
        })
    return nc, in_maps, meta


def _assemble(results, meta):
    outs = [np.ascontiguousarray(results[c]["out"].T)     # [Bc, C]
            for c in range(N_CORES)]
    return np.concatenate(outs, axis=0).astype(np.float32)


def kernel(**inputs):
    nc, in_maps, meta = _prepare(**inputs)
    res = bass_utils.run_bass_kernel_spmd(nc, in_maps,
                                          core_ids=list(range(N_CORES)))
    return _assemble(res.results, meta)


def kernel_traced(tmpdir=None, **inputs):
    """Same as kernel() but with NTFF profiling; returns (output, results)."""
    nc, in_maps, meta = _prepare(**inputs)
    res = bass_utils.run_bass_kernel_spmd(nc, in_maps,
                                          core_ids=list(range(N_CORES)),
                                          trace=True, tmpdir=tmpdir)
    return _assemble(res.results, meta), res


# revision 37
# speedup vs baseline: 1.2195x; 1.2195x over previous
"""Trainium2 Bass kernel for the ButterflyMlp problem.

Computes log_softmax(L3(relu(L2(relu(L1(x)))))) where each Li is a masked
linear layer (butterfly sparsity: global column stripes + a diagonal band),
batch 65536, data-parallel over 8 NeuronCores (8192 rows/core).

Strategy (per core, feature-major throughout):
  - L1 exploits butterfly structure: stripe columns S (204, dense for every
    output row) are one shared K-axis; each 112-row output block adds a
    narrow band residual (<=92 cols). Both run as fp8e4 DoubleRow matmuls
    (two 128-row K-planes per pass, 0.5 cycles/col): 2 matmuls per block.
  - L1 bias is folded into the stripe matmul via an appended ones-row in the
    packed x (weight row = b1), so PSUM evictions are pure ReLU+cast ops,
    split round-robin across ScalarE/DVE/Pool.
  - L2 (K=784) runs as 3 fp8 DoubleRow pair matmuls over paired y1 tiles
    plus one fp16 tail matmul; bias b2 via ACT on the y2 eviction.
  - L3 (K=128, C=10) is fp16 with C padded to 32; four consecutive batch
    chunks write one PSUM bank at partition offsets 0/32/64/96 (PE array
    tiling), so exp/ln/subtract of log_softmax batch 4 chunks per pass and
    the logsumexp reduction is a single ones-blockdiag matmul.
  - x is pre-gathered and fp8-packed on host; per-slab DMAs go through
    SWDGE (nc.gpsimd) with 4-28KB contiguous runs per partition; all xs
    slabs are prefetched ahead of the larger xb slabs. Outputs stream back
    on the sync HWDGE ring.
"""
import sys
sys.path.insert(0, "/opt/trn_rl_repo")
import numpy as np
import ml_dtypes

import concourse.bass as bass
import concourse.bacc as bacc
import concourse.mybir as mybir
import concourse.tile as tile
from concourse import bass_utils

F32 = mybir.dt.float32
F16 = mybir.dt.float16
F8 = mybir.dt.float8e4
NP8 = ml_dtypes.float8_e4m3
AF = mybir.ActivationFunctionType
ALU = mybir.AluOpType
DR = mybir.MatmulPerfMode.DoubleRow

# Keep every ACT function this kernel uses (Relu/Exp/Ln + implicit Copy /
# Identity) inside one activation-table set so the greedy per-function set
# chooser emits a single table load instead of reloading per chunk.
_PIN_SET = "natural_log_exp_and_others"
_orig_gat = bacc.get_activation_tables


def _pinned_gat(arch):
    tabs = _orig_gat(arch)
    need = {AF.Relu, AF.Identity, AF.Exp, AF.Ln, AF.Copy}
    if _PIN_SET in tabs and need <= tabs[_PIN_SET]:
        for name in tabs:
            if name != _PIN_SET:
                tabs[name] = tabs[name] - need
    return tabs


bacc.get_activation_tables = _pinned_gat

N_CORES = 8
NB = 512          # batch columns per matmul (one fp32 PSUM bank)
SC = 2048         # batch columns per DMA slab
OT = 112          # L1 output block width (784/7)
GROUP = 3         # batch chunks per L3/log-softmax group (3*32 = 96 rows;
                  # AP base partitions are limited to 0/32/64)
CP = 32           # padded class count (PE tile col granularity)


def _decompose_mask1(mask1):
    D_out, D_in = mask1.shape
    S = np.where(mask1.all(axis=0))[0]
    n_blk = (D_out + OT - 1) // OT
    stripe_set = np.zeros(D_in, dtype=bool)
    stripe_set[S] = True
    R_list = []
    for j in range(n_blk):
        blk = mask1[j * OT:(j + 1) * OT]
        R_list.append(np.where(blk.any(axis=0) & ~stripe_set)[0])
    return S, R_list


def _build_program(meta):
    PS, PB = meta["PS"], meta["PB"]
    Bc = meta["Bc"]
    D1, H, C = meta["D1"], meta["H"], meta["C"]
    n_blk = D1 // OT
    n_pair = n_blk // 2                 # L2 DoubleRow pairs (tail is fp16)
    n_sup = Bc // SC
    n_chunk = Bc // NB

    nc = bacc.Bacc("TRN2", target_bir_lowering=False, debug=False,
                   enable_asserts=False, num_devices=N_CORES)

    xs_d = nc.dram_tensor("xs", [PS, n_sup * 2 * SC], F8,
                          kind="ExternalInput").ap()
    xb_d = nc.dram_tensor("xb", [PB, n_sup * n_blk * SC], F8,
                          kind="ExternalInput").ap()
    ws_d = nc.dram_tensor("ws", [PS, n_blk * 2 * OT], F8,
                          kind="ExternalInput").ap()
    wb_d = nc.dram_tensor("wb", [PB, n_blk * OT], F8,
                          kind="ExternalInput").ap()
    w2_d = nc.dram_tensor("w2", [OT, n_blk * H], F8,
                          kind="ExternalInput").ap()
    w3_d = nc.dram_tensor("w3", [H, CP], F16, kind="ExternalInput").ap()
    ones_d = nc.dram_tensor("ones", [GROUP * CP, GROUP * CP], F16,
                            kind="ExternalInput").ap()
    b2_d = nc.dram_tensor("b2", [H, 1], F32, kind="ExternalInput").ap()
    b3t_d = nc.dram_tensor("b3t", [GROUP * CP, 1], F32,
                           kind="ExternalInput").ap()
    out_d = nc.dram_tensor("out", [C, Bc], F32, kind="ExternalOutput").ap()

    with tile.TileContext(nc) as tc:
        with tc.tile_pool(name="wp", bufs=1) as wp, \
             tc.tile_pool(name="xsp", bufs=n_sup) as xsp, \
             tc.tile_pool(name="xbp", bufs=2) as xbp, \
             tc.tile_pool(name="y1p", bufs=15) as y1pp, \
             tc.tile_pool(name="y2p", bufs=4) as y2p, \
             tc.tile_pool(name="exp", bufs=2) as exp_p, \
             tc.tile_pool(name="y3p", bufs=2) as y3p, \
             tc.tile_pool(name="lsp", bufs=2) as lsp, \
             tc.tile_pool(name="op", bufs=2) as op, \
             tc.tile_pool(name="ps1", bufs=5, space="PSUM") as ps1, \
             tc.tile_pool(name="ps2", bufs=1, space="PSUM") as ps2, \
             tc.tile_pool(name="ps34", bufs=2, space="PSUM") as ps34:

            # ---- resident weights (SWDGE, ahead of the x slabs) ----
            ws_sb = wp.tile([PS, n_blk, 2, OT], F8)
            nc.gpsimd.dma_start(ws_sb[:], ws_d[:])
            wb_sb = wp.tile([PB, n_blk, OT], F8)
            nc.gpsimd.dma_start(wb_sb[:], wb_d[:])
            w2_sb = wp.tile([OT, n_blk, H], F8)
            nc.gpsimd.dma_start(w2_sb[:], w2_d[:])
            w3_sb = wp.tile([H, CP], F16)
            nc.gpsimd.dma_start(w3_sb[:], w3_d[:])
            ones_sb = wp.tile([GROUP * CP, GROUP * CP], F16)
            nc.gpsimd.dma_start(ones_sb[:], ones_d[:])
            b2_sb = wp.tile([H, 1], F32)
            nc.sync.dma_start(b2_sb[:], b2_d[:])
            b3t_sb = wp.tile([GROUP * CP, 1], F32)
            nc.sync.dma_start(b3t_sb[:], b3t_d[:])

            # ---- x slab loads: all xs first (small), then xb per slab.
            # Chunk-major layout: every per-chunk moving slice is fully
            # contiguous (the PE's fast path needs packed moving APs).
            n_half = SC // NB
            xs_tiles = []
            for s in range(n_sup):
                t = xsp.tile([PS, n_half, 2, NB], F8, name=f"xs{s}", tag="xs")
                nc.gpsimd.dma_start(t[:], xs_d[:, s * 2 * SC:(s + 1) * 2 * SC])
                xs_tiles.append(t)
            xb_tiles = []
            for s in range(n_sup):
                t = xbp.tile([PB, n_half, n_blk, NB], F8,
                             name=f"xb{s}", tag="xb")
                nc.gpsimd.dma_start(
                    t[:], xb_d[:, s * n_blk * SC:(s + 1) * n_blk * SC])
                xb_tiles.append(t)

            # Per-chunk state threaded through the software pipeline.
            y1_tiles = {}    # c -> [7 y1 tiles]
            p2_tiles = {}    # c -> L2 PSUM tile
            y2_tiles = {}    # c -> y2 SBUF tile
            p3_tiles = {}    # g -> L3 group PSUM tile
            ex_tiles = {}    # g -> exp SBUF tile
            y3_tiles = {}    # g -> logits+bias SBUF tile
            pending_tails = []  # (g, body_ready) awaiting softmax tail
            gsize = lambda g: min(GROUP, n_chunk - g * GROUP)

            def emit_l2_all(c):
                # The whole L2 for chunk c, deferred one body: every y1
                # eviction is long done, so the PE never waits here. The y2
                # eviction is NOT emitted here: it would head the epilogue
                # queue and block this body's L1 evictions behind the L2
                # chain. The body emits it after its second L1 eviction.
                p2_tiles[c] = ps2.tile([H, NB], F32, tag="l2", name="p2")
                for k in range(n_blk):
                    nc.tensor.matmul(p2_tiles[c][:], w2_sb[:, k, :],
                                     y1_tiles[c][k][:], start=(k == 0),
                                     stop=(k == n_blk - 1))
                del y1_tiles[c]
                emit_y2_evict(c)

            def emit_y2_evict(c):
                y2 = y2p.tile([H, NB], F16, tag="y2")
                if c % 2 == 0:
                    nc.scalar.activation(y2[:], p2_tiles[c][:], AF.Relu,
                                         bias=b2_sb[:, 0:1])
                else:
                    nc.vector.tensor_scalar(y2[:], p2_tiles[c][:],
                                            b2_sb[:, 0:1], 0.0,
                                            op0=ALU.add, op1=ALU.max)
                y2_tiles[c] = y2
                del p2_tiles[c]

            def emit_l3_group(g, body):
                # All of the group's col-32 L3 matmuls back to back: the PE
                # column-tile reconfig (128 -> 32 -> 128) is paid once per
                # group instead of once per chunk.
                gs = gsize(g)
                gp = gs * CP
                p3 = ps34.tile([GROUP * CP, NB], F32, tag="l34", name="p3")
                for m in range(gs):
                    c = g * GROUP + m
                    nc.tensor.matmul(p3[m * CP:(m + 1) * CP, :],
                                     w3_sb[:], y2_tiles[c][:],
                                     start=True, stop=True)
                    del y2_tiles[c]
                p3_tiles[g] = p3

            def emit_exp(g, body):
                # Exp + bias-add for a finished group — placed at body end so
                # the epilogue engines drain this body's evictions first.
                gs = gsize(g)
                gp = gs * CP
                p3 = p3_tiles.pop(g)
                ex = exp_p.tile([GROUP * CP, NB], F16, tag="ex")
                nc.scalar.activation(ex[:gp, :], p3[:gp, :],
                                     AF.Exp, bias=b3t_sb[:gp, 0:1])
                y3 = y3p.tile([GROUP * CP, NB], F32, tag="y3")
                nc.vector.tensor_scalar(y3[:gp, :], p3[:gp, :],
                                        b3t_sb[:gp, 0:1], None, op0=ALU.add)
                ex_tiles[g] = ex
                y3_tiles[g] = y3
                pending_tails.append((g, body))

            def emit_softmax_tail(g):
                gs = gsize(g)
                gp = gs * CP
                psl = ps34.tile([GROUP * CP, NB], F32, tag="l34", name="psl")
                nc.tensor.matmul(psl[:gp, :], ones_sb[:gp, :gp],
                                 ex_tiles[g][:gp, :], start=True, stop=True)
                ls = lsp.tile([GROUP * CP, NB], F32, tag="ls")
                nc.scalar.activation(ls[:gp, :], psl[:gp, :], AF.Ln)
                o = op.tile([GROUP * CP, NB], F32, tag="o")
                nc.gpsimd.tensor_tensor(o[:gp, :], y3_tiles[g][:gp, :],
                                        ls[:gp, :], op=ALU.subtract)
                for m in range(gs):
                    cc = g * GROUP + m
                    nc.sync.dma_start(out_d[:, cc * NB:(cc + 1) * NB],
                                      o[m * CP:m * CP + C, :])
                del ex_tiles[g], y3_tiles[g]

            def flush_tails(body):
                while pending_tails and pending_tails[0][1] < body:
                    emit_softmax_tail(pending_tails.pop(0)[0])

            def l1_mms(dst_psum, j, xs_t, xb_t, h):
                nc.tensor.matmul(dst_psum, ws_sb[:, j, 0, :],
                                 xs_t[:, h, 0, :],
                                 start=True, stop=False)
                nc.tensor.matmul(dst_psum, ws_sb[:, j, 1, :],
                                 xs_t[:, h, 1, :],
                                 start=False, stop=False)
                nc.tensor.matmul(dst_psum, wb_sb[:, j, :],
                                 xb_t[:, h, j, :],
                                 start=False, stop=True)

            for c in range(n_chunk):
                s, h = c // (SC // NB), c % (SC // NB)
                xs_t, xb_t = xs_tiles[s], xb_tiles[s]

                # Deferred work from earlier chunks heads the PE stream:
                # their data deps resolved most of a body ago, so no stalls.
                if c >= 1:
                    emit_l2_all(c - 1)
                if c >= 2 and ((c - 2) % GROUP == GROUP - 1
                               or c - 2 == n_chunk - 1):
                    emit_l3_group((c - 2) // GROUP, c)

                # Seven L1 blocks, each three fp8 matmuls into one PSUM
                # bank, evicted by a pure ReLU+cast alternating ACT/DVE.
                y1s = []
                for j in range(n_blk):
                    p1 = ps1.tile([OT, NB], F32, tag="l1", name="p1")
                    l1_mms(p1[:], j, xs_t, xb_t, h)
                    y1 = y1pp.tile([OT, NB], F8, tag="y1")
                    if j % 2 == 0:
                        nc.scalar.activation(y1[:], p1[:], AF.Relu)
                    else:
                        nc.vector.tensor_scalar(y1[:], p1[:], 0.0, None,
                                                op0=ALU.max)
                    y1s.append(y1)
                y1_tiles[c] = y1s

                # Group tail work last: the epilogue engines finish this
                # body's evictions before touching exp/ln/subtract.
                if c >= 2 and ((c - 2) % GROUP == GROUP - 1
                               or c - 2 == n_chunk - 1):
                    emit_exp((c - 2) // GROUP, c)
                flush_tails(c)

            # ---- drain the pipeline ----
            emit_l2_all(n_chunk - 1)
            for cc in (n_chunk - 2, n_chunk - 1):
                if cc % GROUP == GROUP - 1 or cc == n_chunk - 1:
                    g = cc // GROUP
                    emit_l3_group(g, n_chunk)
                    emit_exp(g, n_chunk)
            for g, _ in list(pending_tails):
                emit_softmax_tail(g)
            pending_tails.clear()

    nc.compile()
    return nc


_CACHE = {}


def _prepare(x, W1, b1, W2, b2, W3, b3, mask1, mask2, mask3):
    B, D1 = x.shape
    H = W2.shape[0]
    C = W3.shape[0]
    assert B % N_CORES == 0
    Bc = B // N_CORES
    n_blk = D1 // OT
    n_pair = n_blk // 2
    n_sup = Bc // SC

    S, R_list = _decompose_mask1(np.asarray(mask1))
    nS = len(S)
    PS = (nS + 2 + 1) // 2              # stripe K-planes incl ones+zero rows
    maxR = max(len(r) for r in R_list)
    # >=65 partitions keeps the PE in its 128-row tile config: mixing 64-row
    # and 128-row matmuls in one stream costs a reconfig bubble per matmul.
    PB = max(maxR, 65)

    Wm1 = (np.asarray(W1) * np.asarray(mask1)).astype(np.float32)
    Wm2 = (np.asarray(W2) * np.asarray(mask2)).astype(np.float32)
    Wm3 = (np.asarray(W3) * np.asarray(mask3)).astype(np.float32)
    b1 = np.asarray(b1, np.float32)
    b2 = np.asarray(b2, np.float32)
    b3 = np.asarray(b3, np.float32)

    c8 = lambda a: np.asarray(a, dtype=NP8)
    c16 = lambda a: np.asarray(a, dtype=np.float16)

    # ---- stripe pack: K order = S cols, then ones row, then zero pad ----
    xT = np.asarray(x, np.float32).T                     # [D1, B]
    SP2 = 2 * PS
    xs_src = np.zeros((SP2, B), np.float32)
    xs_src[:nS] = xT[S]
    xs_src[nS] = 1.0
    # [NC, PS, n_sup, n_half, 2, NB] — chunk-major for contiguous slices
    n_half = SC // NB
    xs8 = (c8(xs_src).reshape(2, PS, N_CORES, n_sup, n_half, NB)
           .transpose(2, 1, 3, 4, 0, 5))
    xs8 = np.ascontiguousarray(xs8.reshape(N_CORES, PS, n_sup * 2 * SC))

    Ws_full = np.zeros((D1, SP2), np.float32)
    Ws_full[:, :nS] = Wm1[:, S]
    Ws_full[:, nS] = b1
    ws8 = np.ascontiguousarray(
        c8(Ws_full).reshape(n_blk, OT, 2, PS).transpose(3, 0, 2, 1)
        .reshape(PS, n_blk * 2 * OT))

    # ---- band pack (flat K, fp8 single matmuls) ----
    xb_src = np.zeros((n_blk, PB, B), np.float32)
    wb_src = np.zeros((n_blk, OT, PB), np.float32)
    for j, R in enumerate(R_list):
        xb_src[j, :len(R)] = xT[R]
        wb_src[j, :, :len(R)] = Wm1[j * OT:(j + 1) * OT, R]
    # [NC, PB, n_sup, n_half, n_blk, NB]
    xb8 = (c8(xb_src).reshape(n_blk, PB, N_CORES, n_sup, n_half, NB)
           .transpose(2, 1, 3, 4, 0, 5))
    xb8 = np.ascontiguousarray(
        xb8.reshape(N_CORES, PB, n_sup * n_blk * SC))
    wb8 = np.ascontiguousarray(
        c8(wb_src).transpose(2, 0, 1).reshape(PB, n_blk * OT))

    # ---- L2 pack: seven fp8 single matmuls ----
    t2 = Wm2.T.reshape(n_blk, OT, H)                     # [j, p, h]
    w2a8 = np.ascontiguousarray(
        c8(t2).transpose(1, 0, 2).reshape(OT, n_blk * H))

    # ---- L3 pack: classes padded to CP, 4 chunks per PE-tile group ----
    w3p = np.zeros((H, CP), np.float32)
    w3p[:, :C] = Wm3.T
    w3p16 = c16(w3p)
    GC = GROUP * CP
    ones = np.zeros((GC, GC), np.float32)
    for g in range(GROUP):
        ones[g * CP:g * CP + C, g * CP:g * CP + C] = 1.0
    ones16 = c16(ones)
    b3t = np.zeros((GC, 1), np.float32)
    for g in range(GROUP):
        b3t[g * CP:g * CP + C, 0] = b3
    b2p = b2.reshape(H, 1)

    meta = dict(PS=PS, PB=PB, Bc=Bc, D1=D1, H=H, C=C)
    key = (B, D1, H, C, nS, PB)
    if key not in _CACHE:
        _CACHE[key] = _build_program(meta)
    nc = _CACHE[key]

    in_maps = []
    for c in range(N_CORES):
        in_maps.append({
            "xs": xs8[c], "xb": xb8[c],
            "ws": ws8, "wb": wb8, "w2": w2a8,
            "w3": w3p16, "ones": ones16, "b2": b2p, "b3t": b3t,
        })
    return nc, in_maps, meta


def _assemble(results, meta):
    outs = [np.ascontiguousarray(results[c]["out"].T)     # [Bc, C]
            for c in range(N_CORES)]
    return np.concatenate(outs, axis=0).astype(np.float32)


def kernel(**inputs):
    nc, in_maps, meta = _prepare(**inputs)
    res = bass_utils.run_bass_kernel_spmd(nc, in_maps,
                                          core_ids=list(range(N_CORES)))
    return _assemble(res.results, meta)


def kernel_traced(tmpdir=None, **inputs):
    """Same as kernel() but with NTFF profiling; returns (output, results)."""
    nc, in_maps, meta = _prepare(**inputs)
    res = bass_utils.run_bass_kernel_spmd(nc, in_maps,
                                          core_ids=list(range(N_CORES)),
                                          trace=True, tmpdir=tmpdir)
    return _assemble(res.results, meta), res


# revision 38
# speedup vs baseline: 1.2219x; 1.0020x over previous
"""Trainium2 Bass kernel for the ButterflyMlp problem.

Computes log_softmax(L3(relu(L2(relu(L1(x)))))) where each Li is a masked
linear layer (butterfly sparsity: global column stripes + a diagonal band),
batch 65536, data-parallel over 8 NeuronCores (8192 rows/core).

Strategy (per core, feature-major, batch chunks of 512 columns):
  - L1 exploits butterfly structure: stripe columns S (204, dense for every
    output row) are a shared K-axis split in two 103-row planes; each
    112-row output block adds a narrow band residual (<=92 cols). All
    matmuls are fp8e4 single-row mode: on this platform, 8 cores running
    fp8 DoubleRow trigger a chip-level clock throttle (~1.4GHz) that
    exactly cancels DoubleRow's 2x K-throughput, while single-row fp8/fp16
    streams sustain the full 2.4GHz (1 moving column/cycle).
  - Every matmul keeps >=65 K-partitions so the PE stays in its 128-row
    tile config; mixing 64-row and 128-row tiles costs a reconfig bubble
    per matmul (~1.7x slowdown measured).
  - L1 bias is folded into the stripe matmul via an appended ones-row in
    the packed x (weight row = b1), so PSUM evictions are pure ReLU+cast
    ops alternating ScalarE/DVE (Pool cannot read PSUM).
  - Each chunk's L2 (7 fp8 matmuls, K=112) is deferred one chunk so its y1
    evictions are long done when the PE reaches it; b2 is applied by the
    y2 eviction (ACT bias / DVE tensor_scalar, alternating parity).
  - L3 (K=128, fp16) pads classes 10->32; three consecutive chunks write
    one PSUM bank at partition offsets 0/32/64 (PE column tiling, emitted
    back to back to amortize the col-32 reconfig), so exp/ln/subtract of
    log_softmax run once per 3 chunks and logsumexp is a single
    ones-blockdiag fp16 matmul. exp/+bias read PSUM directly; the final
    subtract runs on GpSimd (SBUF only).
  - x is pre-gathered and fp8-packed on host in chunk-major layout so all
    moving APs are contiguous; per-slab SWDGE DMAs (4-24KB rows), all xs
    slabs prefetched ahead of the larger xb slabs; outputs stream back on
    the sync HWDGE ring, 3 chunks per transfer.
"""
import sys
sys.path.insert(0, "/opt/trn_rl_repo")
import numpy as np
import ml_dtypes

import concourse.bass as bass
import concourse.bacc as bacc
import concourse.mybir as mybir
import concourse.tile as tile
from concourse import bass_utils

F32 = mybir.dt.float32
F16 = mybir.dt.float16
F8 = mybir.dt.float8e4
NP8 = ml_dtypes.float8_e4m3
AF = mybir.ActivationFunctionType
ALU = mybir.AluOpType
DR = mybir.MatmulPerfMode.DoubleRow

# Keep every ACT function this kernel uses (Relu/Exp/Ln + implicit Copy /
# Identity) inside one activation-table set so the greedy per-function set
# chooser emits a single table load instead of reloading per chunk.
_PIN_SET = "natural_log_exp_and_others"
_orig_gat = bacc.get_activation_tables


def _pinned_gat(arch):
    tabs = _orig_gat(arch)
    need = {AF.Relu, AF.Identity, AF.Exp, AF.Ln, AF.Copy}
    if _PIN_SET in tabs and need <= tabs[_PIN_SET]:
        for name in tabs:
            if name != _PIN_SET:
                tabs[name] = tabs[name] - need
    return tabs


bacc.get_activation_tables = _pinned_gat

N_CORES = 8
NB = 512          # batch columns per matmul (one fp32 PSUM bank)
SC = 2048         # batch columns per DMA slab
OT = 112          # L1 output block width (784/7)
GROUP = 3         # batch chunks per L3/log-softmax group (3*32 = 96 rows;
                  # AP base partitions are limited to 0/32/64)
CP = 32           # padded class count (PE tile col granularity)


def _decompose_mask1(mask1):
    D_out, D_in = mask1.shape
    S = np.where(mask1.all(axis=0))[0]
    n_blk = (D_out + OT - 1) // OT
    stripe_set = np.zeros(D_in, dtype=bool)
    stripe_set[S] = True
    R_list = []
    for j in range(n_blk):
        blk = mask1[j * OT:(j + 1) * OT]
        R_list.append(np.where(blk.any(axis=0) & ~stripe_set)[0])
    return S, R_list


def _build_program(meta):
    PS, PB = meta["PS"], meta["PB"]
    Bc = meta["Bc"]
    D1, H, C = meta["D1"], meta["H"], meta["C"]
    n_blk = D1 // OT
    n_pair = n_blk // 2                 # L2 DoubleRow pairs (tail is fp16)
    n_sup = Bc // SC
    n_chunk = Bc // NB

    nc = bacc.Bacc("TRN2", target_bir_lowering=False, debug=False,
                   enable_asserts=False, num_devices=N_CORES)

    xs_d = nc.dram_tensor("xs", [PS, n_sup * 2 * SC], F8,
                          kind="ExternalInput").ap()
    xb_d = nc.dram_tensor("xb", [PB, n_sup * n_blk * SC], F8,
                          kind="ExternalInput").ap()
    ws_d = nc.dram_tensor("ws", [PS, n_blk * 2 * OT], F8,
                          kind="ExternalInput").ap()
    wb_d = nc.dram_tensor("wb", [PB, n_blk * OT], F8,
                          kind="ExternalInput").ap()
    w2_d = nc.dram_tensor("w2", [OT, n_blk * H], F8,
                          kind="ExternalInput").ap()
    w3_d = nc.dram_tensor("w3", [H, CP], F16, kind="ExternalInput").ap()
    ones_d = nc.dram_tensor("ones", [GROUP * CP, GROUP * CP], F16,
                            kind="ExternalInput").ap()
    b2_d = nc.dram_tensor("b2", [H, 1], F32, kind="ExternalInput").ap()
    b3t_d = nc.dram_tensor("b3t", [GROUP * CP, 1], F32,
                           kind="ExternalInput").ap()
    out_d = nc.dram_tensor("out", [C, Bc], F32, kind="ExternalOutput").ap()

    with tile.TileContext(nc) as tc:
        with tc.tile_pool(name="wp", bufs=1) as wp, \
             tc.tile_pool(name="xsp", bufs=n_sup) as xsp, \
             tc.tile_pool(name="xbp", bufs=2) as xbp, \
             tc.tile_pool(name="y1p", bufs=15) as y1pp, \
             tc.tile_pool(name="y2p", bufs=4) as y2p, \
             tc.tile_pool(name="exp", bufs=2) as exp_p, \
             tc.tile_pool(name="y3p", bufs=2) as y3p, \
             tc.tile_pool(name="lsp", bufs=2) as lsp, \
             tc.tile_pool(name="op", bufs=2) as op, \
             tc.tile_pool(name="ps1", bufs=5, space="PSUM") as ps1, \
             tc.tile_pool(name="ps2", bufs=1, space="PSUM") as ps2, \
             tc.tile_pool(name="ps34", bufs=2, space="PSUM") as ps34:

            # ---- resident weights (SWDGE, ahead of the x slabs) ----
            ws_sb = wp.tile([PS, n_blk, 2, OT], F8)
            nc.gpsimd.dma_start(ws_sb[:], ws_d[:])
            wb_sb = wp.tile([PB, n_blk, OT], F8)
            nc.gpsimd.dma_start(wb_sb[:], wb_d[:])
            w2_sb = wp.tile([OT, n_blk, H], F8)
            nc.gpsimd.dma_start(w2_sb[:], w2_d[:])
            w3_sb = wp.tile([H, CP], F16)
            nc.gpsimd.dma_start(w3_sb[:], w3_d[:])
            ones_sb = wp.tile([GROUP * CP, GROUP * CP], F16)
            nc.gpsimd.dma_start(ones_sb[:], ones_d[:])
            b2_sb = wp.tile([H, 1], F32)
            nc.sync.dma_start(b2_sb[:], b2_d[:])
            b3t_sb = wp.tile([GROUP * CP, 1], F32)
            nc.sync.dma_start(b3t_sb[:], b3t_d[:])

            # ---- x slab loads: all xs first (small), then xb per slab.
            # Chunk-major layout: every per-chunk moving slice is fully
            # contiguous (the PE's fast path needs packed moving APs).
            n_half = SC // NB
            xs_tiles = []
            for s in range(n_sup):
                t = xsp.tile([PS, n_half, 2, NB], F8, name=f"xs{s}", tag="xs")
                nc.gpsimd.dma_start(t[:], xs_d[:, s * 2 * SC:(s + 1) * 2 * SC])
                xs_tiles.append(t)
            xb_tiles = []
            for s in range(n_sup):
                t = xbp.tile([PB, n_half, n_blk, NB], F8,
                             name=f"xb{s}", tag="xb")
                nc.gpsimd.dma_start(
                    t[:], xb_d[:, s * n_blk * SC:(s + 1) * n_blk * SC])
                xb_tiles.append(t)

            # Per-chunk state threaded through the software pipeline.
            y1_tiles = {}    # c -> [7 y1 tiles]
            p2_tiles = {}    # c -> L2 PSUM tile
            y2_tiles = {}    # c -> y2 SBUF tile
            p3_tiles = {}    # g -> L3 group PSUM tile
            ex_tiles = {}    # g -> exp SBUF tile
            y3_tiles = {}    # g -> logits+bias SBUF tile
            pending_tails = []  # (g, body_ready) awaiting softmax tail
            gsize = lambda g: min(GROUP, n_chunk - g * GROUP)

            def emit_l2_all(c):
                # The whole L2 for chunk c, deferred one body: every y1
                # eviction is long done, so the PE never waits here. The y2
                # eviction is NOT emitted here: it would head the epilogue
                # queue and block this body's L1 evictions behind the L2
                # chain. The body emits it after its second L1 eviction.
                p2_tiles[c] = ps2.tile([H, NB], F32, tag="l2", name="p2")
                for k in range(n_blk):
                    nc.tensor.matmul(p2_tiles[c][:], w2_sb[:, k, :],
                                     y1_tiles[c][k][:], start=(k == 0),
                                     stop=(k == n_blk - 1))
                del y1_tiles[c]
                emit_y2_evict(c)

            def emit_y2_evict(c):
                y2 = y2p.tile([H, NB], F16, tag="y2")
                if c % 2 == 0:
                    nc.scalar.activation(y2[:], p2_tiles[c][:], AF.Relu,
                                         bias=b2_sb[:, 0:1])
                else:
                    nc.vector.tensor_scalar(y2[:], p2_tiles[c][:],
                                            b2_sb[:, 0:1], 0.0,
                                            op0=ALU.add, op1=ALU.max)
                y2_tiles[c] = y2
                del p2_tiles[c]

            def emit_l3_group(g, body):
                # All of the group's col-32 L3 matmuls back to back: the PE
                # column-tile reconfig (128 -> 32 -> 128) is paid once per
                # group instead of once per chunk.
                gs = gsize(g)
                gp = gs * CP
                p3 = ps34.tile([GROUP * CP, NB], F32, tag="l34", name="p3")
                for m in range(gs):
                    c = g * GROUP + m
                    nc.tensor.matmul(p3[m * CP:(m + 1) * CP, :],
                                     w3_sb[:], y2_tiles[c][:],
                                     start=True, stop=True)
                    del y2_tiles[c]
                p3_tiles[g] = p3

            def emit_exp(g, body):
                # Exp + bias-add for a finished group — placed at body end so
                # the epilogue engines drain this body's evictions first.
                gs = gsize(g)
                gp = gs * CP
                p3 = p3_tiles.pop(g)
                ex = exp_p.tile([GROUP * CP, NB], F16, tag="ex")
                nc.scalar.activation(ex[:gp, :], p3[:gp, :],
                                     AF.Exp, bias=b3t_sb[:gp, 0:1])
                y3 = y3p.tile([GROUP * CP, NB], F32, tag="y3")
                nc.vector.tensor_scalar(y3[:gp, :], p3[:gp, :],
                                        b3t_sb[:gp, 0:1], None, op0=ALU.add)
                ex_tiles[g] = ex
                y3_tiles[g] = y3
                pending_tails.append((g, body))

            def emit_softmax_tail(g):
                gs = gsize(g)
                gp = gs * CP
                psl = ps34.tile([GROUP * CP, NB], F32, tag="l34", name="psl")
                nc.tensor.matmul(psl[:gp, :], ones_sb[:gp, :gp],
                                 ex_tiles[g][:gp, :], start=True, stop=True)
                ls = lsp.tile([GROUP * CP, NB], F32, tag="ls")
                nc.scalar.activation(ls[:gp, :], psl[:gp, :], AF.Ln)
                o = op.tile([GROUP * CP, NB], F32, tag="o")
                nc.gpsimd.tensor_tensor(o[:gp, :], y3_tiles[g][:gp, :],
                                        ls[:gp, :], op=ALU.subtract)
                for m in range(gs):
                    cc = g * GROUP + m
                    nc.sync.dma_start(out_d[:, cc * NB:(cc + 1) * NB],
                                      o[m * CP:m * CP + C, :])
                del ex_tiles[g], y3_tiles[g]

            def flush_tails(body):
                while pending_tails and pending_tails[0][1] < body:
                    emit_softmax_tail(pending_tails.pop(0)[0])

            def l1_mms(dst_psum, j, xs_t, xb_t, h):
                nc.tensor.matmul(dst_psum, ws_sb[:, j, 0, :],
                                 xs_t[:, h, 0, :],
                                 start=True, stop=False)
                nc.tensor.matmul(dst_psum, ws_sb[:, j, 1, :],
                                 xs_t[:, h, 1, :],
                                 start=False, stop=False)
                nc.tensor.matmul(dst_psum, wb_sb[:, j, :],
                                 xb_t[:, h, j, :],
                                 start=False, stop=True)

            for c in range(n_chunk):
                s, h = c // (SC // NB), c % (SC // NB)
                xs_t, xb_t = xs_tiles[s], xb_tiles[s]

                # Deferred work from earlier chunks heads the PE stream:
                # their data deps resolved most of a body ago, so no stalls.
                if c >= 1:
                    emit_l2_all(c - 1)
                if c >= 2 and ((c - 2) % GROUP == GROUP - 1
                               or c - 2 == n_chunk - 1):
                    emit_l3_group((c - 2) // GROUP, c)

                # Seven L1 blocks, each three fp8 matmuls into one PSUM
                # bank, evicted by a pure ReLU+cast alternating ACT/DVE.
                y1s = []
                for j in range(n_blk):
                    p1 = ps1.tile([OT, NB], F32, tag="l1", name="p1")
                    l1_mms(p1[:], j, xs_t, xb_t, h)
                    y1 = y1pp.tile([OT, NB], F8, tag="y1")
                    if j % 2 == 0:
                        nc.scalar.activation(y1[:], p1[:], AF.Relu)
                    else:
                        nc.vector.tensor_scalar(y1[:], p1[:], 0.0, None,
                                                op0=ALU.max)
                    y1s.append(y1)
                y1_tiles[c] = y1s

                # Group tail work last: the epilogue engines finish this
                # body's evictions before touching exp/ln/subtract.
                if c >= 2 and ((c - 2) % GROUP == GROUP - 1
                               or c - 2 == n_chunk - 1):
                    emit_exp((c - 2) // GROUP, c)
                flush_tails(c)

            # ---- drain the pipeline ----
            emit_l2_all(n_chunk - 1)
            for cc in (n_chunk - 2, n_chunk - 1):
                if cc % GROUP == GROUP - 1 or cc == n_chunk - 1:
                    g = cc // GROUP
                    emit_l3_group(g, n_chunk)
                    emit_exp(g, n_chunk)
            for g, _ in list(pending_tails):
                emit_softmax_tail(g)
            pending_tails.clear()

    nc.compile()
    return nc


_CACHE = {}


def _prepare(x, W1, b1, W2, b2, W3, b3, mask1, mask2, mask3):
    B, D1 = x.shape
    H = W2.shape[0]
    C = W3.shape[0]
    assert B % N_CORES == 0
    Bc = B // N_CORES
    n_blk = D1 // OT
    n_pair = n_blk // 2
    n_sup = Bc // SC

    S, R_list = _decompose_mask1(np.asarray(mask1))
    nS = len(S)
    PS = (nS + 2 + 1) // 2              # stripe K-planes incl ones+zero rows
    maxR = max(len(r) for r in R_list)
    # >=65 partitions keeps the PE in its 128-row tile config: mixing 64-row
    # and 128-row matmuls in one stream costs a reconfig bubble per matmul.
    PB = max(maxR, 65)

    Wm1 = (np.asarray(W1) * np.asarray(mask1)).astype(np.float32)
    Wm2 = (np.asarray(W2) * np.asarray(mask2)).astype(np.float32)
    Wm3 = (np.asarray(W3) * np.asarray(mask3)).astype(np.float32)
    b1 = np.asarray(b1, np.float32)
    b2 = np.asarray(b2, np.float32)
    b3 = np.asarray(b3, np.float32)

    c8 = lambda a: np.asarray(a, dtype=NP8)
    c16 = lambda a: np.asarray(a, dtype=np.float16)

    # ---- stripe pack: K order = S cols, then ones row, then zero pad ----
    xT = np.asarray(x, np.float32).T                     # [D1, B]
    SP2 = 2 * PS
    xs_src = np.zeros((SP2, B), np.float32)
    xs_src[:nS] = xT[S]
    xs_src[nS] = 1.0
    # [NC, PS, n_sup, n_half, 2, NB] — chunk-major for contiguous slices
    n_half = SC // NB
    xs8 = (c8(xs_src).reshape(2, PS, N_CORES, n_sup, n_half, NB)
           .transpose(2, 1, 3, 4, 0, 5))
    xs8 = np.ascontiguousarray(xs8.reshape(N_CORES, PS, n_sup * 2 * SC))

    Ws_full = np.zeros((D1, SP2), np.float32)
    Ws_full[:, :nS] = Wm1[:, S]
    Ws_full[:, nS] = b1
    ws8 = np.ascontiguousarray(
        c8(Ws_full).reshape(n_blk, OT, 2, PS).transpose(3, 0, 2, 1)
        .reshape(PS, n_blk * 2 * OT))

    # ---- band pack (flat K, fp8 single matmuls) ----
    xb_src = np.zeros((n_blk, PB, B), np.float32)
    wb_src = np.zeros((n_blk, OT, PB), np.float32)
    for j, R in enumerate(R_list):
        xb_src[j, :len(R)] = xT[R]
        wb_src[j, :, :len(R)] = Wm1[j * OT:(j + 1) * OT, R]
    # [NC, PB, n_sup, n_half, n_blk, NB]
    xb8 = (c8(xb_src).reshape(n_blk, PB, N_CORES, n_sup, n_half, NB)
           .transpose(2, 1, 3, 4, 0, 5))
    xb8 = np.ascontiguousarray(
        xb8.reshape(N_CORES, PB, n_sup * n_blk * SC))
    wb8 = np.ascontiguousarray(
        c8(wb_src).transpose(2, 0, 1).reshape(PB, n_blk * OT))

    # ---- L2 pack: seven fp8 single matmuls ----
    t2 = Wm2.T.reshape(n_blk, OT, H)                     # [j, p, h]
    w2a8 = np.ascontiguousarray(
        c8(t2).transpose(1, 0, 2).reshape(OT, n_blk * H))

    # ---- L3 pack: classes padded to CP, 4 chunks per PE-tile group ----
    w3p = np.zeros((H, CP), np.float32)
    w3p[:, :C] = Wm3.T
    w3p16 = c16(w3p)
    GC = GROUP * CP
    ones = np.zeros((GC, GC), np.float32)
    for g in range(GROUP):
        ones[g * CP:g * CP + C, g * CP:g * CP + C] = 1.0
    ones16 = c16(ones)
    b3t = np.zeros((GC, 1), np.float32)
    for g in range(GROUP):
        b3t[g * CP:g * CP + C, 0] = b3
    b2p = b2.reshape(H, 1)

    meta = dict(PS=PS, PB=PB, Bc=Bc, D1=D1, H=H, C=C)
    key = (B, D1, H, C, nS, PB)
    if key not in _CACHE:
        _CACHE[key] = _build_program(meta)
    nc = _CACHE[key]

    in_maps = []
    for c in range(N_CORES):
        in_maps.append({
            "xs": xs8[c], "xb": xb8[c],
            "ws": ws8, "wb": wb8, "w2": w2a8,
            "w3": w3p16, "ones": ones16, "b2": b2p, "b3t": b3t,
        })
    return nc, in_maps, meta


def _assemble(results, meta):
    outs = [np.ascontiguousarray(results[c]["out"].T)     # [Bc, C]
            for c in range(N_CORES)]
    return np.concatenate(outs, axis=0).astype(np.float32)


def kernel(**inputs):
    nc, in_maps, meta = _prepare(**inputs)
    res = bass_utils.run_bass_kernel_spmd(nc, in_maps,
                                          core_ids=list(range(N_CORES)))
    return _assemble(res.results, meta)


def kernel_traced(tmpdir=None, **inputs):
    """Same as kernel() but with NTFF profiling; returns (output, results)."""
    nc, in_maps, meta = _prepare(**inputs)
    res = bass_utils.run_bass_kernel_spmd(nc, in_maps,
                                          core_ids=list(range(N_CORES)),
                                          trace=True, tmpdir=tmpdir)
    return _assemble(res.results, meta), res


# revision 39
# speedup vs baseline: 1.3827x; 1.1316x over previous
"""Trainium2 Bass kernel for the ButterflyMlp problem.

Computes log_softmax(L3(relu(L2(relu(L1(x)))))) where each Li is a masked
linear layer (butterfly sparsity: global column stripes + a diagonal band),
batch 65536, data-parallel over 8 NeuronCores (8192 rows/core).

Strategy (per core, feature-major, batch chunks of 512 columns):
  - L1 exploits butterfly structure: stripe columns S (204, dense for every
    output row) are a shared K-axis split in two 103-row planes; each
    112-row output block adds a narrow band residual (<=92 cols). All
    matmuls are fp8e4 single-row mode: on this platform, 8 cores running
    fp8 DoubleRow trigger a chip-level clock throttle (~1.4GHz) that
    exactly cancels DoubleRow's 2x K-throughput, while single-row fp8/fp16
    streams sustain the full 2.4GHz (1 moving column/cycle).
  - Every matmul keeps >=65 K-partitions so the PE stays in its 128-row
    tile config; mixing 64-row and 128-row tiles costs a reconfig bubble
    per matmul (~1.7x slowdown measured).
  - L1 bias is folded into the stripe matmul via an appended ones-row in
    the packed x (weight row = b1), so PSUM evictions are pure ReLU+cast
    ops alternating ScalarE/DVE (Pool cannot read PSUM).
  - Each chunk's L2 (7 fp8 matmuls, K=112) is deferred one chunk so its y1
    evictions are long done when the PE reaches it; b2 is applied by the
    y2 eviction (ACT bias / DVE tensor_scalar, alternating parity).
  - L3 (K=128, fp16) pads classes 10->32; three consecutive chunks write
    one PSUM bank at partition offsets 0/32/64 (PE column tiling, emitted
    back to back to amortize the col-32 reconfig), so exp/ln/subtract of
    log_softmax run once per 3 chunks and logsumexp is a single
    ones-blockdiag fp16 matmul. exp/+bias read PSUM directly; the final
    subtract runs on GpSimd (SBUF only).
  - x is pre-gathered and fp8-packed on host in chunk-major layout so all
    moving APs are contiguous; per-slab SWDGE DMAs (4-24KB rows), all xs
    slabs prefetched ahead of the larger xb slabs; outputs stream back on
    the sync HWDGE ring, 3 chunks per transfer.
"""
import sys
sys.path.insert(0, "/opt/trn_rl_repo")
import numpy as np
import ml_dtypes

import concourse.bass as bass
import concourse.bacc as bacc
import concourse.mybir as mybir
import concourse.tile as tile
from concourse import bass_utils

F32 = mybir.dt.float32
F16 = mybir.dt.float16
F8 = mybir.dt.float8e4
NP8 = ml_dtypes.float8_e4m3
AF = mybir.ActivationFunctionType
ALU = mybir.AluOpType
DR = mybir.MatmulPerfMode.DoubleRow

# Keep every ACT function this kernel uses (Relu/Exp/Ln + implicit Copy /
# Identity) inside one activation-table set so the greedy per-function set
# chooser emits a single table load instead of reloading per chunk.
_PIN_SET = "natural_log_exp_and_others"
_orig_gat = bacc.get_activation_tables


def _pinned_gat(arch):
    tabs = _orig_gat(arch)
    need = {AF.Relu, AF.Identity, AF.Exp, AF.Ln, AF.Copy}
    if _PIN_SET in tabs and need <= tabs[_PIN_SET]:
        for name in tabs:
            if name != _PIN_SET:
                tabs[name] = tabs[name] - need
    return tabs


bacc.get_activation_tables = _pinned_gat

N_CORES = 8
NB = 512          # batch columns per matmul (one fp32 PSUM bank)
SC = 2048         # batch columns per DMA slab
OT = 112          # L1 output block width (784/7)
GROUP = 3         # batch chunks per L3/log-softmax group (3*32 = 96 rows;
                  # AP base partitions are limited to 0/32/64)
CP = 32           # padded class count (PE tile col granularity)


def _decompose_mask1(mask1):
    D_out, D_in = mask1.shape
    S = np.where(mask1.all(axis=0))[0]
    n_blk = (D_out + OT - 1) // OT
    stripe_set = np.zeros(D_in, dtype=bool)
    stripe_set[S] = True
    R_list = []
    for j in range(n_blk):
        blk = mask1[j * OT:(j + 1) * OT]
        R_list.append(np.where(blk.any(axis=0) & ~stripe_set)[0])
    return S, R_list


def _build_program(meta):
    PS, PB = meta["PS"], meta["PB"]
    Bc = meta["Bc"]
    D1, H, C = meta["D1"], meta["H"], meta["C"]
    n_blk = D1 // OT
    n_pair = n_blk // 2                 # L2 DoubleRow pairs (tail is fp16)
    n_sup = Bc // SC
    n_chunk = Bc // NB

    nc = bacc.Bacc("TRN2", target_bir_lowering=False, debug=False,
                   enable_asserts=False, num_devices=N_CORES)

    xs_d = nc.dram_tensor("xs", [PS, n_sup * 2 * SC], F8,
                          kind="ExternalInput").ap()
    xb_d = nc.dram_tensor("xb", [PB, n_sup * n_blk * SC], F8,
                          kind="ExternalInput").ap()
    ws_d = nc.dram_tensor("ws", [PS, n_blk * 2 * OT], F8,
                          kind="ExternalInput").ap()
    wb_d = nc.dram_tensor("wb", [PB, n_blk * OT], F8,
                          kind="ExternalInput").ap()
    w2_d = nc.dram_tensor("w2", [OT, n_blk * H], F8,
                          kind="ExternalInput").ap()
    w3_d = nc.dram_tensor("w3", [H, CP], F16, kind="ExternalInput").ap()
    ones_d = nc.dram_tensor("ones", [GROUP * CP, GROUP * CP], F16,
                            kind="ExternalInput").ap()
    b2_d = nc.dram_tensor("b2", [H, 1], F32, kind="ExternalInput").ap()
    b3t_d = nc.dram_tensor("b3t", [GROUP * CP, 1], F32,
                           kind="ExternalInput").ap()
    out_d = nc.dram_tensor("out", [C, Bc], F32, kind="ExternalOutput").ap()

    with tile.TileContext(nc) as tc:
        with tc.tile_pool(name="wp", bufs=1) as wp, \
             tc.tile_pool(name="xsp", bufs=n_sup) as xsp, \
             tc.tile_pool(name="xbp", bufs=2) as xbp, \
             tc.tile_pool(name="y1p", bufs=15) as y1pp, \
             tc.tile_pool(name="y2p", bufs=4) as y2p, \
             tc.tile_pool(name="exp", bufs=2) as exp_p, \
             tc.tile_pool(name="y3p", bufs=2) as y3p, \
             tc.tile_pool(name="lsp", bufs=2) as lsp, \
             tc.tile_pool(name="op", bufs=2) as op, \
             tc.tile_pool(name="ps1", bufs=5, space="PSUM") as ps1, \
             tc.tile_pool(name="ps2", bufs=1, space="PSUM") as ps2, \
             tc.tile_pool(name="ps34", bufs=2, space="PSUM") as ps34:

            # ---- resident weights (SWDGE, ahead of the x slabs) ----
            ws_sb = wp.tile([PS, n_blk, 2, OT], F8)
            nc.scalar.dma_start(ws_sb[:], ws_d[:])
            wb_sb = wp.tile([PB, n_blk, OT], F8)
            nc.sync.dma_start(wb_sb[:], wb_d[:])
            w2_sb = wp.tile([OT, n_blk, H], F8)
            nc.scalar.dma_start(w2_sb[:], w2_d[:])
            w3_sb = wp.tile([H, CP], F16)
            nc.sync.dma_start(w3_sb[:], w3_d[:])
            ones_sb = wp.tile([GROUP * CP, GROUP * CP], F16)
            nc.sync.dma_start(ones_sb[:], ones_d[:])
            b2_sb = wp.tile([H, 1], F32)
            nc.sync.dma_start(b2_sb[:], b2_d[:])
            b3t_sb = wp.tile([GROUP * CP, 1], F32)
            nc.sync.dma_start(b3t_sb[:], b3t_d[:])

            # ---- x slab loads: all xs first (small), then xb per slab.
            # Chunk-major layout: every per-chunk moving slice is fully
            # contiguous (the PE's fast path needs packed moving APs).
            n_half = SC // NB
            xs_tiles, xb_tiles = [], []
            for s in range(n_sup):
                xs_tiles.append(xsp.tile([PS, n_half, 2, NB], F8,
                                         name=f"xs{s}", tag="xs"))
                xb_tiles.append(xbp.tile([PB, n_half, n_blk, NB], F8,
                                         name=f"xb{s}", tag="xb"))
            # Slab 0 streams in per-chunk pieces so the first body's data
            # (~430KB) arrives long before the whole slab; later slabs load
            # whole, interleaved xs-then-xb (the cold DMA path runs at a
            # fraction of its steady rate, so first bytes matter most).
            xsw, xbw = 2 * NB, n_blk * NB
            for h in range(n_half):
                nc.gpsimd.dma_start(xs_tiles[0][:, h, :, :],
                                    xs_d[:, h * xsw:(h + 1) * xsw])
                nc.gpsimd.dma_start(xb_tiles[0][:, h, :, :],
                                    xb_d[:, h * xbw:(h + 1) * xbw])
            for s in range(1, n_sup):
                nc.gpsimd.dma_start(
                    xs_tiles[s][:], xs_d[:, s * 2 * SC:(s + 1) * 2 * SC])
                nc.gpsimd.dma_start(
                    xb_tiles[s][:], xb_d[:, s * n_blk * SC:(s + 1) * n_blk * SC])

            # Per-chunk state threaded through the software pipeline.
            y1_tiles = {}    # c -> [7 y1 tiles]
            p2_tiles = {}    # c -> L2 PSUM tile
            y2_tiles = {}    # c -> y2 SBUF tile
            p3_tiles = {}    # g -> L3 group PSUM tile
            ex_tiles = {}    # g -> exp SBUF tile
            y3_tiles = {}    # g -> logits+bias SBUF tile
            pending_tails = []  # (g, body_ready) awaiting softmax tail
            gsize = lambda g: min(GROUP, n_chunk - g * GROUP)

            def emit_l2_all(c):
                # The whole L2 for chunk c, deferred one body: every y1
                # eviction is long done, so the PE never waits here. The y2
                # eviction is NOT emitted here: it would head the epilogue
                # queue and block this body's L1 evictions behind the L2
                # chain. The body emits it after its second L1 eviction.
                p2_tiles[c] = ps2.tile([H, NB], F32, tag="l2", name="p2")
                for k in range(n_blk):
                    nc.tensor.matmul(p2_tiles[c][:], w2_sb[:, k, :],
                                     y1_tiles[c][k][:], start=(k == 0),
                                     stop=(k == n_blk - 1))
                del y1_tiles[c]
                emit_y2_evict(c)

            def emit_y2_evict(c):
                y2 = y2p.tile([H, NB], F16, tag="y2")
                if c % 2 == 0:
                    nc.scalar.activation(y2[:], p2_tiles[c][:], AF.Relu,
                                         bias=b2_sb[:, 0:1])
                else:
                    nc.vector.tensor_scalar(y2[:], p2_tiles[c][:],
                                            b2_sb[:, 0:1], 0.0,
                                            op0=ALU.add, op1=ALU.max)
                y2_tiles[c] = y2
                del p2_tiles[c]

            def emit_l3_group(g, body):
                # All of the group's col-32 L3 matmuls back to back: the PE
                # column-tile reconfig (128 -> 32 -> 128) is paid once per
                # group instead of once per chunk.
                gs = gsize(g)
                gp = gs * CP
                p3 = ps34.tile([GROUP * CP, NB], F32, tag="l34", name="p3")
                for m in range(gs):
                    c = g * GROUP + m
                    nc.tensor.matmul(p3[m * CP:(m + 1) * CP, :],
                                     w3_sb[:], y2_tiles[c][:],
                                     start=True, stop=True)
                    del y2_tiles[c]
                p3_tiles[g] = p3

            def emit_exp(g, body):
                # Exp + bias-add for a finished group — placed at body end so
                # the epilogue engines drain this body's evictions first.
                gs = gsize(g)
                gp = gs * CP
                p3 = p3_tiles.pop(g)
                ex = exp_p.tile([GROUP * CP, NB], F16, tag="ex")
                nc.scalar.activation(ex[:gp, :], p3[:gp, :],
                                     AF.Exp, bias=b3t_sb[:gp, 0:1])
                y3 = y3p.tile([GROUP * CP, NB], F32, tag="y3")
                nc.vector.tensor_scalar(y3[:gp, :], p3[:gp, :],
                                        b3t_sb[:gp, 0:1], None, op0=ALU.add)
                ex_tiles[g] = ex
                y3_tiles[g] = y3
                pending_tails.append((g, body))

            def emit_softmax_tail(g):
                gs = gsize(g)
                gp = gs * CP
                psl = ps34.tile([GROUP * CP, NB], F32, tag="l34", name="psl")
                nc.tensor.matmul(psl[:gp, :], ones_sb[:gp, :gp],
                                 ex_tiles[g][:gp, :], start=True, stop=True)
                ls = lsp.tile([GROUP * CP, NB], F32, tag="ls")
                nc.scalar.activation(ls[:gp, :], psl[:gp, :], AF.Ln)
                o = op.tile([GROUP * CP, NB], F32, tag="o")
                nc.vector.tensor_tensor(o[:gp, :], y3_tiles[g][:gp, :],
                                        ls[:gp, :], op=ALU.subtract)
                for m in range(gs):
                    cc = g * GROUP + m
                    nc.sync.dma_start(out_d[:, cc * NB:(cc + 1) * NB],
                                      o[m * CP:m * CP + C, :])
                del ex_tiles[g], y3_tiles[g]

            def flush_tails(body):
                while pending_tails and pending_tails[0][1] < body:
                    emit_softmax_tail(pending_tails.pop(0)[0])

            def l1_mms(dst_psum, j, xs_t, xb_t, h):
                nc.tensor.matmul(dst_psum, ws_sb[:, j, 0, :],
                                 xs_t[:, h, 0, :],
                                 start=True, stop=False)
                nc.tensor.matmul(dst_psum, ws_sb[:, j, 1, :],
                                 xs_t[:, h, 1, :],
                                 start=False, stop=False)
                nc.tensor.matmul(dst_psum, wb_sb[:, j, :],
                                 xb_t[:, h, j, :],
                                 start=False, stop=True)

            for c in range(n_chunk):
                s, h = c // (SC // NB), c % (SC // NB)
                xs_t, xb_t = xs_tiles[s], xb_tiles[s]

                # Deferred work from earlier chunks heads the PE stream:
                # their data deps resolved most of a body ago, so no stalls.
                if c >= 1:
                    emit_l2_all(c - 1)

                # Seven L1 blocks, each three fp8 matmuls into one PSUM
                # bank, evicted by a pure ReLU+cast alternating ACT/DVE.
                y1s = []
                for j in range(n_blk):
                    p1 = ps1.tile([OT, NB], F32, tag="l1", name="p1")
                    l1_mms(p1[:], j, xs_t, xb_t, h)
                    y1 = y1pp.tile([OT, NB], F8, tag="y1")
                    if j % 2 == 0:
                        nc.scalar.activation(y1[:], p1[:], AF.Relu)
                    else:
                        nc.vector.tensor_scalar(y1[:], p1[:], 0.0, None,
                                                op0=ALU.max)
                    y1s.append(y1)
                y1_tiles[c] = y1s

                # Group tail work last: the epilogue engines finish this
                # body's evictions before touching exp/ln/subtract.
                if c >= 1 and ((c - 1) % GROUP == GROUP - 1
                               or c - 1 == n_chunk - 1):
                    g = (c - 1) // GROUP
                    emit_l3_group(g, c)
                    emit_exp(g, c)
                flush_tails(c)

            # ---- drain the pipeline ----
            emit_l2_all(n_chunk - 1)
            if (n_chunk - 1) % GROUP != GROUP - 1:
                g = (n_chunk - 1) // GROUP
                emit_l3_group(g, n_chunk)
                emit_exp(g, n_chunk)
            for g, _ in list(pending_tails):
                emit_softmax_tail(g)
            pending_tails.clear()

    nc.compile()
    return nc


_CACHE = {}


def _prepare(x, W1, b1, W2, b2, W3, b3, mask1, mask2, mask3):
    B, D1 = x.shape
    H = W2.shape[0]
    C = W3.shape[0]
    assert B % N_CORES == 0
    Bc = B // N_CORES
    n_blk = D1 // OT
    n_pair = n_blk // 2
    n_sup = Bc // SC

    S, R_list = _decompose_mask1(np.asarray(mask1))
    nS = len(S)
    PS = (nS + 2 + 1) // 2              # stripe K-planes incl ones+zero rows
    maxR = max(len(r) for r in R_list)
    # >=65 partitions keeps the PE in its 128-row tile config: mixing 64-row
    # and 128-row matmuls in one stream costs a reconfig bubble per matmul.
    PB = max(maxR, 65)

    Wm1 = (np.asarray(W1) * np.asarray(mask1)).astype(np.float32)
    Wm2 = (np.asarray(W2) * np.asarray(mask2)).astype(np.float32)
    Wm3 = (np.asarray(W3) * np.asarray(mask3)).astype(np.float32)
    b1 = np.asarray(b1, np.float32)
    b2 = np.asarray(b2, np.float32)
    b3 = np.asarray(b3, np.float32)

    c8 = lambda a: np.asarray(a, dtype=NP8)
    c16 = lambda a: np.asarray(a, dtype=np.float16)

    # ---- stripe pack: K order = S cols, then ones row, then zero pad ----
    xT = np.asarray(x, np.float32).T                     # [D1, B]
    SP2 = 2 * PS
    xs_src = np.zeros((SP2, B), np.float32)
    xs_src[:nS] = xT[S]
    xs_src[nS] = 1.0
    # [NC, PS, n_sup, n_half, 2, NB] — chunk-major for contiguous slices
    n_half = SC // NB
    xs8 = (c8(xs_src).reshape(2, PS, N_CORES, n_sup, n_half, NB)
           .transpose(2, 1, 3, 4, 0, 5))
    xs8 = np.ascontiguousarray(xs8.reshape(N_CORES, PS, n_sup * 2 * SC))

    Ws_full = np.zeros((D1, SP2), np.float32)
    Ws_full[:, :nS] = Wm1[:, S]
    Ws_full[:, nS] = b1
    ws8 = np.ascontiguousarray(
        c8(Ws_full).reshape(n_blk, OT, 2, PS).transpose(3, 0, 2, 1)
        .reshape(PS, n_blk * 2 * OT))

    # ---- band pack (flat K, fp8 single matmuls) ----
    xb_src = np.zeros((n_blk, PB, B), np.float32)
    wb_src = np.zeros((n_blk, OT, PB), np.float32)
    for j, R in enumerate(R_list):
        xb_src[j, :len(R)] = xT[R]
        wb_src[j, :, :len(R)] = Wm1[j * OT:(j + 1) * OT, R]
    # [NC, PB, n_sup, n_half, n_blk, NB]
    xb8 = (c8(xb_src).reshape(n_blk, PB, N_CORES, n_sup, n_half, NB)
           .transpose(2, 1, 3, 4, 0, 5))
    xb8 = np.ascontiguousarray(
        xb8.reshape(N_CORES, PB, n_sup * n_blk * SC))
    wb8 = np.ascontiguousarray(
        c8(wb_src).transpose(2, 0, 1).reshape(PB, n_blk * OT))

    # ---- L2 pack: seven fp8 single matmuls ----
    t2 = Wm2.T.reshape(n_blk, OT, H)                     # [j, p, h]
    w2a8 = np.ascontiguousarray(
        c8(t2).transpose(1, 0, 2).reshape(OT, n_blk * H))

    # ---- L3 pack: classes padded to CP, 4 chunks per PE-tile group ----
    w3p = np.zeros((H, CP), np.float32)
    w3p[:, :C] = Wm3.T
    w3p16 = c16(w3p)
    GC = GROUP * CP
    ones = np.zeros((GC, GC), np.float32)
    for g in range(GROUP):
        ones[g * CP:g * CP + C, g * CP:g * CP + C] = 1.0
    ones16 = c16(ones)
    b3t = np.zeros((GC, 1), np.float32)
    for g in range(GROUP):
        b3t[g * CP:g * CP + C, 0] = b3
    b2p = b2.reshape(H, 1)

    meta = dict(PS=PS, PB=PB, Bc=Bc, D1=D1, H=H, C=C)
    key = (B, D1, H, C, nS, PB)
    if key not in _CACHE:
        _CACHE[key] = _build_program(meta)
    nc = _CACHE[key]

    in_maps = []
    for c in range(N_CORES):
        in_maps.append({
            "xs": xs8[c], "xb": xb8[c],
            "ws": ws8, "wb": wb8, "w2": w2a8,
            "w3": w3p16, "ones": ones16, "b2": b2p, "b3t": b3t,
        })
    return nc, in_maps, meta


def _assemble(results, meta):
    outs = [np.ascontiguousarray(results[c]["out"].T)     # [Bc, C]
            for c in range(N_CORES)]
    return np.concatenate(outs, axis=0).astype(np.float32)


def kernel(**inputs):
    nc, in_maps, meta = _prepare(**inputs)
    res = bass_utils.run_bass_kernel_spmd(nc, in_maps,
                                          core_ids=list(range(N_CORES)))
    return _assemble(res.results, meta)


def kernel_traced(tmpdir=None, **inputs):
    """Same as kernel() but with NTFF profiling; returns (output, results)."""
    nc, in_maps, meta = _prepare(**inputs)
    res = bass_utils.run_bass_kernel_spmd(nc, in_maps,
                                          core_ids=list(range(N_CORES)),
                                          trace=True, tmpdir=tmpdir)
    return _assemble(res.results, meta), res


# revision 40
# speedup vs baseline: 1.3920x; 1.0067x over previous
"""Trainium2 Bass kernel for the ButterflyMlp problem.

Computes log_softmax(L3(relu(L2(relu(L1(x)))))) where each Li is a masked
linear layer (butterfly sparsity: global column stripes + a diagonal band),
batch 65536, data-parallel over 8 NeuronCores (8192 rows/core).

Strategy (per core, feature-major, batch chunks of 512 columns):
  - L1 exploits butterfly structure: stripe columns S (204, dense for every
    output row) are a shared K-axis split in two 103-row planes; each
    112-row output block adds a narrow band residual (<=92 cols). All
    matmuls are fp8e4 single-row mode: on this platform, 8 cores running
    fp8 DoubleRow trigger a chip-level clock throttle (~1.4GHz) that
    exactly cancels DoubleRow's 2x K-throughput, while single-row fp8/fp16
    streams sustain the full 2.4GHz (1 moving column/cycle).
  - Every matmul keeps >=65 K-partitions so the PE stays in its 128-row
    tile config; mixing 64-row and 128-row tiles costs a reconfig bubble
    per matmul (~1.7x slowdown measured).
  - L1 bias is folded into the stripe matmul via an appended ones-row in
    the packed x (weight row = b1), so PSUM evictions are pure ReLU+cast
    ops alternating ScalarE/DVE (Pool cannot read PSUM).
  - Each chunk's L2 (7 fp8 matmuls, K=112) is deferred one chunk so its y1
    evictions are long done when the PE reaches it; b2 is applied by the
    y2 eviction (ACT bias / DVE tensor_scalar, alternating parity).
  - L3 (K=128, fp16) pads classes 10->32; three consecutive chunks write
    one PSUM bank at partition offsets 0/32/64 (PE column tiling, emitted
    back to back to amortize the col-32 reconfig), so exp/ln/subtract of
    log_softmax run once per 3 chunks and logsumexp is a single
    ones-blockdiag fp16 matmul. exp/+bias read PSUM directly; the final
    subtract runs on GpSimd (SBUF only).
  - x is pre-gathered and fp8-packed on host in chunk-major layout so all
    moving APs are contiguous; per-slab SWDGE DMAs (4-24KB rows), all xs
    slabs prefetched ahead of the larger xb slabs; outputs stream back on
    the sync HWDGE ring, 3 chunks per transfer.
"""
import sys
sys.path.insert(0, "/opt/trn_rl_repo")
import numpy as np
import ml_dtypes

import concourse.bass as bass
import concourse.bacc as bacc
import concourse.mybir as mybir
import concourse.tile as tile
from concourse import bass_utils

F32 = mybir.dt.float32
F16 = mybir.dt.float16
F8 = mybir.dt.float8e4
NP8 = ml_dtypes.float8_e4m3
AF = mybir.ActivationFunctionType
ALU = mybir.AluOpType
DR = mybir.MatmulPerfMode.DoubleRow

# Keep every ACT function this kernel uses (Relu/Exp/Ln + implicit Copy /
# Identity) inside one activation-table set so the greedy per-function set
# chooser emits a single table load instead of reloading per chunk.
_PIN_SET = "natural_log_exp_and_others"
_orig_gat = bacc.get_activation_tables


def _pinned_gat(arch):
    tabs = _orig_gat(arch)
    need = {AF.Relu, AF.Identity, AF.Exp, AF.Ln, AF.Copy}
    if _PIN_SET in tabs and need <= tabs[_PIN_SET]:
        for name in tabs:
            if name != _PIN_SET:
                tabs[name] = tabs[name] - need
    return tabs


bacc.get_activation_tables = _pinned_gat

N_CORES = 8
NB = 512          # batch columns per matmul (one fp32 PSUM bank)
SC = 2048         # batch columns per DMA slab
OT = 112          # L1 output block width (784/7)
GROUP = 3         # batch chunks per L3/log-softmax group (3*32 = 96 rows;
                  # AP base partitions are limited to 0/32/64)
CP = 32           # padded class count (PE tile col granularity)


def _decompose_mask1(mask1):
    D_out, D_in = mask1.shape
    S = np.where(mask1.all(axis=0))[0]
    n_blk = (D_out + OT - 1) // OT
    stripe_set = np.zeros(D_in, dtype=bool)
    stripe_set[S] = True
    R_list = []
    for j in range(n_blk):
        blk = mask1[j * OT:(j + 1) * OT]
        R_list.append(np.where(blk.any(axis=0) & ~stripe_set)[0])
    return S, R_list


def _build_program(meta):
    PS, PB = meta["PS"], meta["PB"]
    Bc = meta["Bc"]
    D1, H, C = meta["D1"], meta["H"], meta["C"]
    n_blk = D1 // OT
    n_pair = n_blk // 2                 # L2 DoubleRow pairs (tail is fp16)
    n_sup = Bc // SC
    n_chunk = Bc // NB

    nc = bacc.Bacc("TRN2", target_bir_lowering=False, debug=False,
                   enable_asserts=False, num_devices=N_CORES)

    xs_d = nc.dram_tensor("xs", [PS, n_sup * 2 * SC], F8,
                          kind="ExternalInput").ap()
    xb_d = nc.dram_tensor("xb", [PB, n_sup * n_blk * SC], F8,
                          kind="ExternalInput").ap()
    ws_d = nc.dram_tensor("ws", [PS, n_blk * 2 * OT], F8,
                          kind="ExternalInput").ap()
    wb_d = nc.dram_tensor("wb", [PB, n_blk * OT], F8,
                          kind="ExternalInput").ap()
    w2_d = nc.dram_tensor("w2", [OT, n_blk * H], F8,
                          kind="ExternalInput").ap()
    w3_d = nc.dram_tensor("w3", [H, CP], F16, kind="ExternalInput").ap()
    ones_d = nc.dram_tensor("ones", [GROUP * CP, GROUP * CP], F16,
                            kind="ExternalInput").ap()
    b2_d = nc.dram_tensor("b2", [H, 1], F32, kind="ExternalInput").ap()
    b3t_d = nc.dram_tensor("b3t", [GROUP * CP, 1], F32,
                           kind="ExternalInput").ap()
    out_d = nc.dram_tensor("out", [C, Bc], F32, kind="ExternalOutput").ap()

    with tile.TileContext(nc) as tc:
        with tc.tile_pool(name="wp", bufs=1) as wp, \
             tc.tile_pool(name="xsp", bufs=n_sup) as xsp, \
             tc.tile_pool(name="xbp", bufs=2) as xbp, \
             tc.tile_pool(name="y1p", bufs=15) as y1pp, \
             tc.tile_pool(name="y2p", bufs=4) as y2p, \
             tc.tile_pool(name="exp", bufs=2) as exp_p, \
             tc.tile_pool(name="y3p", bufs=2) as y3p, \
             tc.tile_pool(name="lsp", bufs=2) as lsp, \
             tc.tile_pool(name="op", bufs=2) as op, \
             tc.tile_pool(name="ps1", bufs=5, space="PSUM") as ps1, \
             tc.tile_pool(name="ps2", bufs=1, space="PSUM") as ps2, \
             tc.tile_pool(name="ps34", bufs=2, space="PSUM") as ps34:

            # ---- resident weights (SWDGE, ahead of the x slabs) ----
            ws_sb = wp.tile([PS, n_blk, 2, OT], F8)
            nc.scalar.dma_start(ws_sb[:], ws_d[:])
            wb_sb = wp.tile([PB, n_blk, OT], F8)
            nc.sync.dma_start(wb_sb[:], wb_d[:])
            w2_sb = wp.tile([OT, n_blk, H], F8)
            nc.scalar.dma_start(w2_sb[:], w2_d[:])
            w3_sb = wp.tile([H, CP], F16)
            nc.sync.dma_start(w3_sb[:], w3_d[:])
            ones_sb = wp.tile([GROUP * CP, GROUP * CP], F16)
            nc.sync.dma_start(ones_sb[:], ones_d[:])
            b2_sb = wp.tile([H, 1], F32)
            nc.sync.dma_start(b2_sb[:], b2_d[:])
            b3t_sb = wp.tile([GROUP * CP, 1], F32)
            nc.sync.dma_start(b3t_sb[:], b3t_d[:])

            # ---- x slab loads: all xs first (small), then xb per slab.
            # Chunk-major layout: every per-chunk moving slice is fully
            # contiguous (the PE's fast path needs packed moving APs).
            n_half = SC // NB
            xs_tiles, xb_tiles = [], []
            for s in range(n_sup):
                xs_tiles.append(xsp.tile([PS, n_half, 2, NB], F8,
                                         name=f"xs{s}", tag="xs"))
                xb_tiles.append(xbp.tile([PB, n_half, n_blk, NB], F8,
                                         name=f"xb{s}", tag="xb"))
            # Slab 0 streams in per-chunk pieces so the first body's data
            # (~430KB) arrives long before the whole slab; later slabs load
            # whole, interleaved xs-then-xb (the cold DMA path runs at a
            # fraction of its steady rate, so first bytes matter most).
            xsw, xbw = 2 * NB, n_blk * NB
            for h in range(n_half):
                nc.gpsimd.dma_start(xs_tiles[0][:, h, :, :],
                                    xs_d[:, h * xsw:(h + 1) * xsw])
                nc.gpsimd.dma_start(xb_tiles[0][:, h, :, :],
                                    xb_d[:, h * xbw:(h + 1) * xbw])
            for s in range(1, n_sup):
                nc.gpsimd.dma_start(
                    xs_tiles[s][:], xs_d[:, s * 2 * SC:(s + 1) * 2 * SC])
                nc.gpsimd.dma_start(
                    xb_tiles[s][:], xb_d[:, s * n_blk * SC:(s + 1) * n_blk * SC])

            # Per-chunk state threaded through the software pipeline.
            y1_tiles = {}    # c -> [7 y1 tiles]
            p2_tiles = {}    # c -> L2 PSUM tile
            y2_tiles = {}    # c -> y2 SBUF tile
            p3_tiles = {}    # g -> L3 group PSUM tile
            ex_tiles = {}    # g -> exp SBUF tile
            y3_tiles = {}    # g -> logits+bias SBUF tile
            pending_tails = []  # (g, body_ready) awaiting softmax tail
            gsize = lambda g: min(GROUP, n_chunk - g * GROUP)

            def emit_l2_all(c):
                # The whole L2 for chunk c, deferred one body: every y1
                # eviction is long done, so the PE never waits here. The y2
                # eviction is NOT emitted here: it would head the epilogue
                # queue and block this body's L1 evictions behind the L2
                # chain. The body emits it after its second L1 eviction.
                p2_tiles[c] = ps2.tile([H, NB], F32, tag="l2", name="p2")
                for k in range(n_blk):
                    nc.tensor.matmul(p2_tiles[c][:], w2_sb[:, k, :],
                                     y1_tiles[c][k][:], start=(k == 0),
                                     stop=(k == n_blk - 1))
                del y1_tiles[c]
                emit_y2_evict(c)

            def emit_y2_evict(c):
                y2 = y2p.tile([H, NB], F16, tag="y2")
                if c % 2 == 0:
                    nc.scalar.activation(y2[:], p2_tiles[c][:], AF.Relu,
                                         bias=b2_sb[:, 0:1])
                else:
                    nc.vector.tensor_scalar(y2[:], p2_tiles[c][:],
                                            b2_sb[:, 0:1], 0.0,
                                            op0=ALU.add, op1=ALU.max)
                y2_tiles[c] = y2
                del p2_tiles[c]

            def emit_l3_group(g, body):
                # All of the group's col-32 L3 matmuls back to back: the PE
                # column-tile reconfig (128 -> 32 -> 128) is paid once per
                # group instead of once per chunk.
                gs = gsize(g)
                gp = gs * CP
                p3 = ps34.tile([GROUP * CP, NB], F32, tag="l34", name="p3")
                for m in range(gs):
                    c = g * GROUP + m
                    nc.tensor.matmul(p3[m * CP:(m + 1) * CP, :],
                                     w3_sb[:], y2_tiles[c][:],
                                     start=True, stop=True)
                    del y2_tiles[c]
                p3_tiles[g] = p3

            def emit_exp(g, body):
                # Exp + bias-add for a finished group — placed at body end so
                # the epilogue engines drain this body's evictions first.
                gs = gsize(g)
                gp = gs * CP
                p3 = p3_tiles.pop(g)
                ex = exp_p.tile([GROUP * CP, NB], F16, tag="ex")
                nc.scalar.activation(ex[:gp, :], p3[:gp, :],
                                     AF.Exp, bias=b3t_sb[:gp, 0:1])
                y3 = y3p.tile([GROUP * CP, NB], F32, tag="y3")
                nc.vector.tensor_scalar(y3[:gp, :], p3[:gp, :],
                                        b3t_sb[:gp, 0:1], None, op0=ALU.add)
                ex_tiles[g] = ex
                y3_tiles[g] = y3
                pending_tails.append((g, body))

            def emit_softmax_tail(g):
                gs = gsize(g)
                gp = gs * CP
                psl = ps34.tile([GROUP * CP, NB], F32, tag="l34", name="psl")
                nc.tensor.matmul(psl[:gp, :], ones_sb[:gp, :gp],
                                 ex_tiles[g][:gp, :], start=True, stop=True)
                ls = lsp.tile([GROUP * CP, NB], F32, tag="ls")
                nc.scalar.activation(ls[:gp, :], psl[:gp, :], AF.Ln)
                o = op.tile([GROUP * CP, NB], F32, tag="o")
                nc.vector.tensor_tensor(o[:gp, :], y3_tiles[g][:gp, :],
                                        ls[:gp, :], op=ALU.subtract)
                ring = nc.scalar if gs < GROUP else nc.sync
                for m in range(gs):
                    cc = g * GROUP + m
                    ring.dma_start(out_d[:, cc * NB:(cc + 1) * NB],
                                   o[m * CP:m * CP + C, :])
                del ex_tiles[g], y3_tiles[g]

            def flush_tails(body):
                while pending_tails and pending_tails[0][1] < body:
                    emit_softmax_tail(pending_tails.pop(0)[0])

            def l1_mms(dst_psum, j, xs_t, xb_t, h):
                nc.tensor.matmul(dst_psum, ws_sb[:, j, 0, :],
                                 xs_t[:, h, 0, :],
                                 start=True, stop=False)
                nc.tensor.matmul(dst_psum, ws_sb[:, j, 1, :],
                                 xs_t[:, h, 1, :],
                                 start=False, stop=False)
                nc.tensor.matmul(dst_psum, wb_sb[:, j, :],
                                 xb_t[:, h, j, :],
                                 start=False, stop=True)

            for c in range(n_chunk):
                s, h = c // (SC // NB), c % (SC // NB)
                xs_t, xb_t = xs_tiles[s], xb_tiles[s]

                # Deferred work from earlier chunks heads the PE stream:
                # their data deps resolved most of a body ago, so no stalls.
                if c >= 1:
                    emit_l2_all(c - 1)

                # Seven L1 blocks, each three fp8 matmuls into one PSUM
                # bank, evicted by a pure ReLU+cast alternating ACT/DVE.
                y1s = []
                for j in range(n_blk):
                    p1 = ps1.tile([OT, NB], F32, tag="l1", name="p1")
                    l1_mms(p1[:], j, xs_t, xb_t, h)
                    y1 = y1pp.tile([OT, NB], F8, tag="y1")
                    if j % 2 == 0:
                        nc.scalar.activation(y1[:], p1[:], AF.Relu)
                    else:
                        nc.vector.tensor_scalar(y1[:], p1[:], 0.0, None,
                                                op0=ALU.max)
                    y1s.append(y1)
                y1_tiles[c] = y1s

                # Group tail work last: the epilogue engines finish this
                # body's evictions before touching exp/ln/subtract.
                if c >= 1 and ((c - 1) % GROUP == GROUP - 1
                               or c - 1 == n_chunk - 1):
                    g = (c - 1) // GROUP
                    emit_l3_group(g, c)
                    emit_exp(g, c)
                flush_tails(c)
                if c == n_chunk - 1:
                    # Start the final chunk's tail chain now: the PE waits
                    # briefly on this body's evictions, but that idle time
                    # would otherwise land in the drain anyway.
                    emit_l2_all(c)
                    g = c // GROUP
                    if c % GROUP != GROUP - 1:
                        emit_l3_group(g, c + 1)
                        emit_exp(g, c + 1)

            # ---- drain the pipeline ----
            for g, _ in list(pending_tails):
                emit_softmax_tail(g)
            pending_tails.clear()

    nc.compile()
    return nc


_CACHE = {}


def _prepare(x, W1, b1, W2, b2, W3, b3, mask1, mask2, mask3):
    B, D1 = x.shape
    H = W2.shape[0]
    C = W3.shape[0]
    assert B % N_CORES == 0
    Bc = B // N_CORES
    n_blk = D1 // OT
    n_pair = n_blk // 2
    n_sup = Bc // SC

    S, R_list = _decompose_mask1(np.asarray(mask1))
    nS = len(S)
    PS = (nS + 2 + 1) // 2              # stripe K-planes incl ones+zero rows
    maxR = max(len(r) for r in R_list)
    # >=65 partitions keeps the PE in its 128-row tile config: mixing 64-row
    # and 128-row matmuls in one stream costs a reconfig bubble per matmul.
    PB = max(maxR, 65)

    Wm1 = (np.asarray(W1) * np.asarray(mask1)).astype(np.float32)
    Wm2 = (np.asarray(W2) * np.asarray(mask2)).astype(np.float32)
    Wm3 = (np.asarray(W3) * np.asarray(mask3)).astype(np.float32)
    b1 = np.asarray(b1, np.float32)
    b2 = np.asarray(b2, np.float32)
    b3 = np.asarray(b3, np.float32)

    c8 = lambda a: np.asarray(a, dtype=NP8)
    c16 = lambda a: np.asarray(a, dtype=np.float16)

    # ---- stripe pack: K order = S cols, then ones row, then zero pad ----
    xT = np.asarray(x, np.float32).T                     # [D1, B]
    SP2 = 2 * PS
    xs_src = np.zeros((SP2, B), np.float32)
    xs_src[:nS] = xT[S]
    xs_src[nS] = 1.0
    # [NC, PS, n_sup, n_half, 2, NB] — chunk-major for contiguous slices
    n_half = SC // NB
    xs8 = (c8(xs_src).reshape(2, PS, N_CORES, n_sup, n_half, NB)
           .transpose(2, 1, 3, 4, 0, 5))
    xs8 = np.ascontiguousarray(xs8.reshape(N_CORES, PS, n_sup * 2 * SC))

    Ws_full = np.zeros((D1, SP2), np.float32)
    Ws_full[:, :nS] = Wm1[:, S]
    Ws_full[:, nS] = b1
    ws8 = np.ascontiguousarray(
        c8(Ws_full).reshape(n_blk, OT, 2, PS).transpose(3, 0, 2, 1)
        .reshape(PS, n_blk * 2 * OT))

    # ---- band pack (flat K, fp8 single matmuls) ----
    xb_src = np.zeros((n_blk, PB, B), np.float32)
    wb_src = np.zeros((n_blk, OT, PB), np.float32)
    for j, R in enumerate(R_list):
        xb_src[j, :len(R)] = xT[R]
        wb_src[j, :, :len(R)] = Wm1[j * OT:(j + 1) * OT, R]
    # [NC, PB, n_sup, n_half, n_blk, NB]
    xb8 = (c8(xb_src).reshape(n_blk, PB, N_CORES, n_sup, n_half, NB)
           .transpose(2, 1, 3, 4, 0, 5))
    xb8 = np.ascontiguousarray(
        xb8.reshape(N_CORES, PB, n_sup * n_blk * SC))
    wb8 = np.ascontiguousarray(
        c8(wb_src).transpose(2, 0, 1).reshape(PB, n_blk * OT))

    # ---- L2 pack: seven fp8 single matmuls ----
    t2 = Wm2.T.reshape(n_blk, OT, H)                     # [j, p, h]
    w2a8 = np.ascontiguousarray(
        c8(t2).transpose(1, 0, 2).reshape(OT, n_blk * H))

    # ---- L3 pack: classes padded to CP, 4 chunks per PE-tile group ----
    w3p = np.zeros((H, CP), np.float32)
    w3p[:, :C] = Wm3.T
    w3p16 = c16(w3p)
    GC = GROUP * CP
    ones = np.zeros((GC, GC), np.float32)
    for g in range(GROUP):
        ones[g * CP:g * CP + C, g * CP:g * CP + C] = 1.0
    ones16 = c16(ones)
    b3t = np.zeros((GC, 1), np.float32)
    for g in range(GROUP):
        b3t[g * CP:g * CP + C, 0] = b3
    b2p = b2.reshape(H, 1)

    meta = dict(PS=PS, PB=PB, Bc=Bc, D1=D1, H=H, C=C)
    key = (B, D1, H, C, nS, PB)
    if key not in _CACHE:
        _CACHE[key] = _build_program(meta)
    nc = _CACHE[key]

    in_maps = []
    for c in range(N_CORES):
        in_maps.append({
            "xs": xs8[c], "xb": xb8[c],
            "ws": ws8, "wb": wb8, "w2": w2a8,
            "w3": w3p16, "ones": ones16, "b2": b2p, "b3t": b3t,
        })
    return nc, in_maps, meta


def _assemble(results, meta):
    outs = [np.ascontiguousarray(results[c]["out"].T)     # [Bc, C]
            for c in range(N_CORES)]
    return np.concatenate(outs, axis=0).astype(np.float32)


def kernel(**inputs):
    nc, in_maps, meta = _prepare(**inputs)
    res = bass_utils.run_bass_kernel_spmd(nc, in_maps,
                                          core_ids=list(range(N_CORES)))
    return _assemble(res.results, meta)


def kernel_traced(tmpdir=None, **inputs):
    """Same as kernel() but with NTFF profiling; returns (output, results)."""
    nc, in_maps, meta = _prepare(**inputs)
    res = bass_utils.run_bass_kernel_spmd(nc, in_maps,
                                          core_ids=list(range(N_CORES)),
                                          trace=True, tmpdir=tmpdir)
    return _assemble(res.results, meta), res


# revision 41
# speedup vs baseline: 1.4757x; 1.0601x over previous
"""Trainium2 Bass kernel for the ButterflyMlp problem.

Computes log_softmax(L3(relu(L2(relu(L1(x)))))) where each Li is a masked
linear layer (butterfly sparsity: global column stripes + a diagonal band),
batch 65536, data-parallel over 8 NeuronCores (8192 rows/core).

Strategy (per core, feature-major, batch chunks of 512 columns):
  - L1 exploits butterfly structure: stripe columns S (204, dense for every
    output row) are a shared K-axis split in two 103-row planes; each
    112-row output block adds a narrow band residual (<=92 cols). All
    matmuls are fp8e4 single-row mode: on this platform, 8 cores running
    fp8 DoubleRow trigger a chip-level clock throttle (~1.4GHz) that
    exactly cancels DoubleRow's 2x K-throughput, while single-row fp8/fp16
    streams sustain the full 2.4GHz (1 moving column/cycle).
  - Every matmul keeps >=65 K-partitions so the PE stays in its 128-row
    tile config; mixing 64-row and 128-row tiles costs a reconfig bubble
    per matmul (~1.7x slowdown measured).
  - L1 bias is folded into the stripe matmul via an appended ones-row in
    the packed x (weight row = b1), so PSUM evictions are pure ReLU+cast
    ops alternating ScalarE/DVE (Pool cannot read PSUM).
  - Each chunk's L2 (7 fp8 matmuls, K=112) is deferred one chunk so its y1
    evictions are long done when the PE reaches it; b2 is applied by the
    y2 eviction (ACT bias / DVE tensor_scalar, alternating parity).
  - L3 (K=128, fp16) pads classes 10->32; three consecutive chunks write
    one PSUM bank at partition offsets 0/32/64 (PE column tiling, emitted
    back to back to amortize the col-32 reconfig), so exp/ln/subtract of
    log_softmax run once per 3 chunks and logsumexp is a single
    ones-blockdiag fp16 matmul. exp/+bias read PSUM directly; the final
    subtract runs on GpSimd (SBUF only).
  - x is pre-gathered and fp8-packed on host in chunk-major layout so all
    moving APs are contiguous; per-slab SWDGE DMAs (4-24KB rows), all xs
    slabs prefetched ahead of the larger xb slabs; outputs stream back on
    the sync HWDGE ring, 3 chunks per transfer.
"""
import sys
sys.path.insert(0, "/opt/trn_rl_repo")
import numpy as np
import ml_dtypes

import concourse.bass as bass
import concourse.bacc as bacc
import concourse.mybir as mybir
import concourse.tile as tile
from concourse import bass_utils

F32 = mybir.dt.float32
F16 = mybir.dt.float16
F8 = mybir.dt.float8e4
NP8 = ml_dtypes.float8_e4m3
AF = mybir.ActivationFunctionType
ALU = mybir.AluOpType
DR = mybir.MatmulPerfMode.DoubleRow

# Keep every ACT function this kernel uses (Relu/Exp/Ln + implicit Copy /
# Identity) inside one activation-table set so the greedy per-function set
# chooser emits a single table load instead of reloading per chunk.
_PIN_SET = "natural_log_exp_and_others"
_orig_gat = bacc.get_activation_tables


def _pinned_gat(arch):
    tabs = _orig_gat(arch)
    need = {AF.Relu, AF.Identity, AF.Exp, AF.Ln, AF.Copy}
    if _PIN_SET in tabs and need <= tabs[_PIN_SET]:
        for name in tabs:
            if name != _PIN_SET:
                tabs[name] = tabs[name] - need
    return tabs


bacc.get_activation_tables = _pinned_gat

N_CORES = 8
NB = 512          # batch columns per matmul (one fp32 PSUM bank)
SC = 2048         # batch columns per DMA slab
OT = 112          # L1 output block width (784/7)
GROUP = 3         # batch chunks per L3/log-softmax group (3*32 = 96 rows;
                  # AP base partitions are limited to 0/32/64)
CP = 32           # padded class count (PE tile col granularity)


def _decompose_mask1(mask1):
    D_out, D_in = mask1.shape
    S = np.where(mask1.all(axis=0))[0]
    n_blk = (D_out + OT - 1) // OT
    stripe_set = np.zeros(D_in, dtype=bool)
    stripe_set[S] = True
    R_list = []
    for j in range(n_blk):
        blk = mask1[j * OT:(j + 1) * OT]
        R_list.append(np.where(blk.any(axis=0) & ~stripe_set)[0])
    return S, R_list


def _build_program(meta):
    PS, PB = meta["PS"], meta["PB"]
    Bc = meta["Bc"]
    D1, H, C = meta["D1"], meta["H"], meta["C"]
    n_blk = D1 // OT
    n_pair = n_blk // 2                 # L2 DoubleRow pairs (tail is fp16)
    n_sup = Bc // SC
    n_chunk = Bc // NB

    nc = bacc.Bacc("TRN2", target_bir_lowering=False, debug=False,
                   enable_asserts=False, num_devices=N_CORES)

    xs_d = nc.dram_tensor("xs", [PS, n_sup * 2 * SC], F8,
                          kind="ExternalInput").ap()
    xb_d = nc.dram_tensor("xb", [PB, n_sup * n_blk * SC], F8,
                          kind="ExternalInput").ap()
    ws_d = nc.dram_tensor("ws", [PS, n_blk * 2 * OT], F8,
                          kind="ExternalInput").ap()
    wb_d = nc.dram_tensor("wb", [PB, n_blk * OT], F8,
                          kind="ExternalInput").ap()
    w2_d = nc.dram_tensor("w2", [OT, n_blk * H], F8,
                          kind="ExternalInput").ap()
    w3_d = nc.dram_tensor("w3", [H, CP], F16, kind="ExternalInput").ap()
    ones_d = nc.dram_tensor("ones", [GROUP * CP, GROUP * CP], F16,
                            kind="ExternalInput").ap()
    b2_d = nc.dram_tensor("b2", [H, 1], F32, kind="ExternalInput").ap()
    b3t_d = nc.dram_tensor("b3t", [GROUP * CP, 1], F32,
                           kind="ExternalInput").ap()
    out_d = nc.dram_tensor("out", [C, Bc], F32, kind="ExternalOutput").ap()

    with tile.TileContext(nc) as tc:
        with tc.tile_pool(name="wp", bufs=1) as wp, \
             tc.tile_pool(name="xsp", bufs=n_sup) as xsp, \
             tc.tile_pool(name="xbp", bufs=2) as xbp, \
             tc.tile_pool(name="y1p", bufs=15) as y1pp, \
             tc.tile_pool(name="y2p", bufs=4) as y2p, \
             tc.tile_pool(name="exp", bufs=2) as exp_p, \
             tc.tile_pool(name="y3p", bufs=2) as y3p, \
             tc.tile_pool(name="lsp", bufs=2) as lsp, \
             tc.tile_pool(name="op", bufs=2) as op, \
             tc.tile_pool(name="ps1", bufs=5, space="PSUM") as ps1, \
             tc.tile_pool(name="ps2", bufs=1, space="PSUM") as ps2, \
             tc.tile_pool(name="ps34", bufs=2, space="PSUM") as ps34:

            # ---- resident weights (SWDGE, ahead of the x slabs) ----
            # ws gates the very first matmul and the cold DMA path costs
            # ~0.18us per partition-row packet: split it across both HWDGE
            # rings so the halves transfer in parallel.
            ws_sb = wp.tile([PS, n_blk, 2, OT], F8)
            wsh = PS // 2
            nc.scalar.dma_start(ws_sb[:wsh], ws_d[:wsh, :])
            nc.sync.dma_start(ws_sb[wsh:], ws_d[wsh:, :])
            wb_sb = wp.tile([PB, n_blk, OT], F8)
            nc.sync.dma_start(wb_sb[:], wb_d[:])
            w2_sb = wp.tile([OT, n_blk, H], F8)
            nc.scalar.dma_start(w2_sb[:], w2_d[:])
            w3_sb = wp.tile([H, CP], F16)
            nc.sync.dma_start(w3_sb[:], w3_d[:])
            ones_sb = wp.tile([GROUP * CP, GROUP * CP], F16)
            nc.sync.dma_start(ones_sb[:], ones_d[:])
            b2_sb = wp.tile([H, 1], F32)
            nc.sync.dma_start(b2_sb[:], b2_d[:])
            b3t_sb = wp.tile([GROUP * CP, 1], F32)
            nc.sync.dma_start(b3t_sb[:], b3t_d[:])

            # ---- x slab loads: all xs first (small), then xb per slab.
            # Chunk-major layout: every per-chunk moving slice is fully
            # contiguous (the PE's fast path needs packed moving APs).
            n_half = SC // NB
            xs_tiles, xb_tiles = [], []
            for s in range(n_sup):
                xs_tiles.append(xsp.tile([PS, n_half, 2, NB], F8,
                                         name=f"xs{s}", tag="xs"))
                xb_tiles.append(xbp.tile([PB, n_half, n_blk, NB], F8,
                                         name=f"xb{s}", tag="xb"))
            # Slab 0 streams in per-chunk pieces so the first body's data
            # (~430KB) arrives long before the whole slab; later slabs load
            # whole, interleaved xs-then-xb (the cold DMA path runs at a
            # fraction of its steady rate, so first bytes matter most).
            xsw, xbw = 2 * NB, n_blk * NB
            for h in range(n_half):
                nc.gpsimd.dma_start(xs_tiles[0][:, h, :, :],
                                    xs_d[:, h * xsw:(h + 1) * xsw])
                nc.gpsimd.dma_start(xb_tiles[0][:, h, :, :],
                                    xb_d[:, h * xbw:(h + 1) * xbw])
            for s in range(1, n_sup):
                nc.gpsimd.dma_start(
                    xs_tiles[s][:], xs_d[:, s * 2 * SC:(s + 1) * 2 * SC])
                nc.gpsimd.dma_start(
                    xb_tiles[s][:], xb_d[:, s * n_blk * SC:(s + 1) * n_blk * SC])

            # Per-chunk state threaded through the software pipeline.
            y1_tiles = {}    # c -> [7 y1 tiles]
            p2_tiles = {}    # c -> L2 PSUM tile
            y2_tiles = {}    # c -> y2 SBUF tile
            p3_tiles = {}    # g -> L3 group PSUM tile
            ex_tiles = {}    # g -> exp SBUF tile
            y3_tiles = {}    # g -> logits+bias SBUF tile
            pending_tails = []  # (g, body_ready) awaiting softmax tail
            gsize = lambda g: min(GROUP, n_chunk - g * GROUP)

            def emit_l2_all(c):
                # The whole L2 for chunk c, deferred one body: every y1
                # eviction is long done, so the PE never waits here. The y2
                # eviction is NOT emitted here: it would head the epilogue
                # queue and block this body's L1 evictions behind the L2
                # chain. The body emits it after its second L1 eviction.
                p2_tiles[c] = ps2.tile([H, NB], F32, tag="l2", name="p2")
                for k in range(n_blk):
                    nc.tensor.matmul(p2_tiles[c][:], w2_sb[:, k, :],
                                     y1_tiles[c][k][:], start=(k == 0),
                                     stop=(k == n_blk - 1))
                del y1_tiles[c]
                emit_y2_evict(c)

            def emit_y2_evict(c):
                y2 = y2p.tile([H, NB], F16, tag="y2")
                if c % 2 == 0:
                    nc.scalar.activation(y2[:], p2_tiles[c][:], AF.Relu,
                                         bias=b2_sb[:, 0:1])
                else:
                    nc.vector.tensor_scalar(y2[:], p2_tiles[c][:],
                                            b2_sb[:, 0:1], 0.0,
                                            op0=ALU.add, op1=ALU.max)
                y2_tiles[c] = y2
                del p2_tiles[c]

            def emit_l3_group(g, body):
                # All of the group's col-32 L3 matmuls back to back: the PE
                # column-tile reconfig (128 -> 32 -> 128) is paid once per
                # group instead of once per chunk.
                gs = gsize(g)
                gp = gs * CP
                p3 = ps34.tile([GROUP * CP, NB], F32, tag="l34", name="p3")
                for m in range(gs):
                    c = g * GROUP + m
                    nc.tensor.matmul(p3[m * CP:(m + 1) * CP, :],
                                     w3_sb[:], y2_tiles[c][:],
                                     start=True, stop=True)
                    del y2_tiles[c]
                p3_tiles[g] = p3

            def emit_exp(g, body):
                # Exp + bias-add for a finished group — placed at body end so
                # the epilogue engines drain this body's evictions first.
                gs = gsize(g)
                gp = gs * CP
                p3 = p3_tiles.pop(g)
                ex = exp_p.tile([GROUP * CP, NB], F16, tag="ex")
                nc.scalar.activation(ex[:gp, :], p3[:gp, :],
                                     AF.Exp, bias=b3t_sb[:gp, 0:1])
                y3 = y3p.tile([GROUP * CP, NB], F32, tag="y3")
                nc.vector.tensor_scalar(y3[:gp, :], p3[:gp, :],
                                        b3t_sb[:gp, 0:1], None, op0=ALU.add)
                ex_tiles[g] = ex
                y3_tiles[g] = y3
                pending_tails.append((g, body))

            def emit_softmax_tail(g):
                gs = gsize(g)
                gp = gs * CP
                psl = ps34.tile([GROUP * CP, NB], F32, tag="l34", name="psl")
                nc.tensor.matmul(psl[:gp, :], ones_sb[:gp, :gp],
                                 ex_tiles[g][:gp, :], start=True, stop=True)
                ls = lsp.tile([GROUP * CP, NB], F32, tag="ls")
                nc.scalar.activation(ls[:gp, :], psl[:gp, :], AF.Ln)
                o = op.tile([GROUP * CP, NB], F32, tag="o")
                nc.vector.tensor_tensor(o[:gp, :], y3_tiles[g][:gp, :],
                                        ls[:gp, :], op=ALU.subtract)
                ring = nc.scalar if gs < GROUP else nc.sync
                for m in range(gs):
                    cc = g * GROUP + m
                    ring.dma_start(out_d[:, cc * NB:(cc + 1) * NB],
                                   o[m * CP:m * CP + C, :])
                del ex_tiles[g], y3_tiles[g]

            def flush_tails(body):
                while pending_tails and pending_tails[0][1] < body:
                    emit_softmax_tail(pending_tails.pop(0)[0])

            def l1_mms(dst_psum, j, xs_t, xb_t, h):
                nc.tensor.matmul(dst_psum, ws_sb[:, j, 0, :],
                                 xs_t[:, h, 0, :],
                                 start=True, stop=False)
                nc.tensor.matmul(dst_psum, ws_sb[:, j, 1, :],
                                 xs_t[:, h, 1, :],
                                 start=False, stop=False)
                nc.tensor.matmul(dst_psum, wb_sb[:, j, :],
                                 xb_t[:, h, j, :],
                                 start=False, stop=True)

            for c in range(n_chunk):
                s, h = c // (SC // NB), c % (SC // NB)
                xs_t, xb_t = xs_tiles[s], xb_tiles[s]

                # Deferred work from earlier chunks heads the PE stream:
                # their data deps resolved most of a body ago, so no stalls.
                if c >= 1:
                    emit_l2_all(c - 1)

                # Seven L1 blocks, each three fp8 matmuls into one PSUM
                # bank, evicted by a pure ReLU+cast alternating ACT/DVE.
                y1s = []
                for j in range(n_blk):
                    p1 = ps1.tile([OT, NB], F32, tag="l1", name="p1")
                    l1_mms(p1[:], j, xs_t, xb_t, h)
                    y1 = y1pp.tile([OT, NB], F8, tag="y1")
                    if j % 2 == 0:
                        nc.scalar.activation(y1[:], p1[:], AF.Relu)
                    else:
                        nc.vector.tensor_scalar(y1[:], p1[:], 0.0, None,
                                                op0=ALU.max)
                    y1s.append(y1)
                y1_tiles[c] = y1s

                # Group tail work last: the epilogue engines finish this
                # body's evictions before touching exp/ln/subtract.
                if c >= 1 and ((c - 1) % GROUP == GROUP - 1
                               or c - 1 == n_chunk - 1):
                    g = (c - 1) // GROUP
                    emit_l3_group(g, c)
                    emit_exp(g, c)
                flush_tails(c)
                if c == n_chunk - 1:
                    # Start the final chunk's tail chain now: the PE waits
                    # briefly on this body's evictions, but that idle time
                    # would otherwise land in the drain anyway.
                    emit_l2_all(c)
                    g = c // GROUP
                    if c % GROUP != GROUP - 1:
                        emit_l3_group(g, c + 1)
                        emit_exp(g, c + 1)

            # ---- drain the pipeline ----
            for g, _ in list(pending_tails):
                emit_softmax_tail(g)
            pending_tails.clear()

    nc.compile()
    return nc


_CACHE = {}


def _prepare(x, W1, b1, W2, b2, W3, b3, mask1, mask2, mask3):
    B, D1 = x.shape
    H = W2.shape[0]
    C = W3.shape[0]
    assert B % N_CORES == 0
    Bc = B // N_CORES
    n_blk = D1 // OT
    n_pair = n_blk // 2
    n_sup = Bc // SC

    S, R_list = _decompose_mask1(np.asarray(mask1))
    nS = len(S)
    PS = (nS + 2 + 1) // 2              # stripe K-planes incl ones+zero rows
    maxR = max(len(r) for r in R_list)
    # >=65 partitions keeps the PE in its 128-row tile config: mixing 64-row
    # and 128-row matmuls in one stream costs a reconfig bubble per matmul.
    PB = max(maxR, 65)

    Wm1 = (np.asarray(W1) * np.asarray(mask1)).astype(np.float32)
    Wm2 = (np.asarray(W2) * np.asarray(mask2)).astype(np.float32)
    Wm3 = (np.asarray(W3) * np.asarray(mask3)).astype(np.float32)
    b1 = np.asarray(b1, np.float32)
    b2 = np.asarray(b2, np.float32)
    b3 = np.asarray(b3, np.float32)

    c8 = lambda a: np.asarray(a, dtype=NP8)
    c16 = lambda a: np.asarray(a, dtype=np.float16)

    # ---- stripe pack: K order = S cols, then ones row, then zero pad ----
    xT = np.asarray(x, np.float32).T                     # [D1, B]
    SP2 = 2 * PS
    xs_src = np.zeros((SP2, B), np.float32)
    xs_src[:nS] = xT[S]
    xs_src[nS] = 1.0
    # [NC, PS, n_sup, n_half, 2, NB] — chunk-major for contiguous slices
    n_half = SC // NB
    xs8 = (c8(xs_src).reshape(2, PS, N_CORES, n_sup, n_half, NB)
           .transpose(2, 1, 3, 4, 0, 5))
    xs8 = np.ascontiguousarray(xs8.reshape(N_CORES, PS, n_sup * 2 * SC))

    Ws_full = np.zeros((D1, SP2), np.float32)
    Ws_full[:, :nS] = Wm1[:, S]
    Ws_full[:, nS] = b1
    ws8 = np.ascontiguousarray(
        c8(Ws_full).reshape(n_blk, OT, 2, PS).transpose(3, 0, 2, 1)
        .reshape(PS, n_blk * 2 * OT))

    # ---- band pack (flat K, fp8 single matmuls) ----
    xb_src = np.zeros((n_blk, PB, B), np.float32)
    wb_src = np.zeros((n_blk, OT, PB), np.float32)
    for j, R in enumerate(R_list):
        xb_src[j, :len(R)] = xT[R]
        wb_src[j, :, :len(R)] = Wm1[j * OT:(j + 1) * OT, R]
    # [NC, PB, n_sup, n_half, n_blk, NB]
    xb8 = (c8(xb_src).reshape(n_blk, PB, N_CORES, n_sup, n_half, NB)
           .transpose(2, 1, 3, 4, 0, 5))
    xb8 = np.ascontiguousarray(
        xb8.reshape(N_CORES, PB, n_sup * n_blk * SC))
    wb8 = np.ascontiguousarray(
        c8(wb_src).transpose(2, 0, 1).reshape(PB, n_blk * OT))

    # ---- L2 pack: seven fp8 single matmuls ----
    t2 = Wm2.T.reshape(n_blk, OT, H)                     # [j, p, h]
    w2a8 = np.ascontiguousarray(
        c8(t2).transpose(1, 0, 2).reshape(OT, n_blk * H))

    # ---- L3 pack: classes padded to CP, 4 chunks per PE-tile group ----
    w3p = np.zeros((H, CP), np.float32)
    w3p[:, :C] = Wm3.T
    w3p16 = c16(w3p)
    GC = GROUP * CP
    ones = np.zeros((GC, GC), np.float32)
    for g in range(GROUP):
        ones[g * CP:g * CP + C, g * CP:g * CP + C] = 1.0
    ones16 = c16(ones)
    b3t = np.zeros((GC, 1), np.float32)
    for g in range(GROUP):
        b3t[g * CP:g * CP + C, 0] = b3
    b2p = b2.reshape(H, 1)

    meta = dict(PS=PS, PB=PB, Bc=Bc, D1=D1, H=H, C=C)
    key = (B, D1, H, C, nS, PB)
    if key not in _CACHE:
        _CACHE[key] = _build_program(meta)
    nc = _CACHE[key]

    in_maps = []
    for c in range(N_CORES):
        in_maps.append({
            "xs": xs8[c], "xb": xb8[c],
            "ws": ws8, "wb": wb8, "w2": w2a8,
            "w3": w3p16, "ones": ones16, "b2": b2p, "b3t": b3t,
        })
    return nc, in_maps, meta


def _assemble(results, meta):
    outs = [np.ascontiguousarray(results[c]["out"].T)     # [Bc, C]
            for c in range(N_CORES)]
    return np.concatenate(outs, axis=0).astype(np.float32)


def kernel(**inputs):
    nc, in_maps, meta = _prepare(**inputs)
    res = bass_utils.run_bass_kernel_spmd(nc, in_maps,
                                          core_ids=list(range(N_CORES)))
    return _assemble(res.results, meta)


def kernel_traced(tmpdir=None, **inputs):
    """Same as kernel() but with NTFF profiling; returns (output, results)."""
    nc, in_maps, meta = _prepare(**inputs)
    res = bass_utils.run_bass_kernel_spmd(nc, in_maps,
                                          core_ids=list(range(N_CORES)),
                                          trace=True, tmpdir=tmpdir)
    return _assemble(res.results, meta), res


# revision 43
# speedup vs baseline: 1.6594x; 1.1245x over previous
"""Trainium2 Bass kernel for the ButterflyMlp problem.

Computes log_softmax(L3(relu(L2(relu(L1(x)))))) where each Li is a masked
linear layer (butterfly sparsity: global column stripes + a diagonal band),
batch 65536, data-parallel over 8 NeuronCores (8192 rows/core).

Strategy (per core, feature-major, batch chunks of 512 columns):
  - L1 exploits butterfly structure: stripe columns S (204, dense for every
    output row) are a shared K-axis split in two 103-row planes; each
    112-row output block adds a narrow band residual (<=92 cols). All
    matmuls are fp8e4 single-row mode: on this platform, 8 cores running
    fp8 DoubleRow trigger a chip-level clock throttle (~1.4GHz) that
    exactly cancels DoubleRow's 2x K-throughput, while single-row fp8/fp16
    streams sustain the full 2.4GHz (1 moving column/cycle).
  - Every matmul keeps >=65 K-partitions so the PE stays in its 128-row
    tile config; mixing 64-row and 128-row tiles costs a reconfig bubble
    per matmul (~1.7x slowdown measured).
  - L1 bias is folded into the stripe matmul via an appended ones-row in
    the packed x (weight row = b1), so PSUM evictions are pure ReLU+cast
    ops alternating ScalarE/DVE (Pool cannot read PSUM).
  - Each chunk's L2 (7 fp8 matmuls, K=112) is deferred one chunk so its y1
    evictions are long done when the PE reaches it; b2 is applied by the
    y2 eviction (ACT bias / DVE tensor_scalar, alternating parity).
  - L3 (K=128, fp16) pads classes 10->32; three consecutive chunks write
    one PSUM bank at partition offsets 0/32/64 (PE column tiling, emitted
    back to back to amortize the col-32 reconfig), so exp/ln/subtract of
    log_softmax run once per 3 chunks and logsumexp is a single
    ones-blockdiag fp16 matmul. exp/+bias read PSUM directly; the final
    subtract runs on DVE.
  - x is pre-gathered and fp8-packed on host in chunk-major layout so all
    moving APs are contiguous. The cold DMA path costs ~0.18us per
    partition-row packet, so startup is packet-count-bound: ws splits
    across both HWDGE rings, slab 0 streams in per-chunk pieces on SWDGE,
    and the weights ride the rings in parallel with the x stream. Outputs
    go back on the sync HWDGE ring (scalar ring for the final ragged
    group), 3 chunks per transfer.
"""
import sys
sys.path.insert(0, "/opt/trn_rl_repo")
import numpy as np
import ml_dtypes

import concourse.bass as bass
import concourse.bacc as bacc
import concourse.mybir as mybir
import concourse.tile as tile
from concourse import bass_utils

F32 = mybir.dt.float32
F16 = mybir.dt.float16
F8 = mybir.dt.float8e4
NP8 = ml_dtypes.float8_e4m3
AF = mybir.ActivationFunctionType
ALU = mybir.AluOpType
DR = mybir.MatmulPerfMode.DoubleRow

# Keep every ACT function this kernel uses (Relu/Exp/Ln + implicit Copy /
# Identity) inside one activation-table set so the greedy per-function set
# chooser emits a single table load instead of reloading per chunk.
_PIN_SET = "natural_log_exp_and_others"
_orig_gat = bacc.get_activation_tables


def _pinned_gat(arch):
    tabs = _orig_gat(arch)
    need = {AF.Relu, AF.Identity, AF.Exp, AF.Ln, AF.Copy}
    if _PIN_SET in tabs and need <= tabs[_PIN_SET]:
        for name in tabs:
            if name != _PIN_SET:
                tabs[name] = tabs[name] - need
    return tabs


bacc.get_activation_tables = _pinned_gat

N_CORES = 8
NB = 512          # batch columns per matmul (one fp32 PSUM bank)
SC = 2048         # batch columns per DMA slab
OT = 112          # L1 output block width (784/7)
GROUP = 3         # batch chunks per L3/log-softmax group (3*32 = 96 rows;
                  # AP base partitions are limited to 0/32/64)
CP = 32           # padded class count (PE tile col granularity)


def _decompose_mask1(mask1):
    D_out, D_in = mask1.shape
    S = np.where(mask1.all(axis=0))[0]
    n_blk = (D_out + OT - 1) // OT
    stripe_set = np.zeros(D_in, dtype=bool)
    stripe_set[S] = True
    R_list = []
    for j in range(n_blk):
        blk = mask1[j * OT:(j + 1) * OT]
        R_list.append(np.where(blk.any(axis=0) & ~stripe_set)[0])
    return S, R_list


def _build_program(meta):
    PS, PB = meta["PS"], meta["PB"]
    Bc = meta["Bc"]
    D1, H, C = meta["D1"], meta["H"], meta["C"]
    n_blk = D1 // OT
    n_pair = n_blk // 2                 # L2 DoubleRow pairs (tail is fp16)
    n_sup = Bc // SC
    n_chunk = Bc // NB

    nc = bacc.Bacc("TRN2", target_bir_lowering=False, debug=False,
                   enable_asserts=False, num_devices=N_CORES)

    xs_d = nc.dram_tensor("xs", [PS, n_sup * 2 * SC], F8,
                          kind="ExternalInput").ap()
    xb_d = nc.dram_tensor("xb", [PB, n_sup * n_blk * SC], F8,
                          kind="ExternalInput").ap()
    ws_d = nc.dram_tensor("ws", [PS, n_blk * 2 * OT], F8,
                          kind="ExternalInput").ap()
    wb_d = nc.dram_tensor("wb", [PB, n_blk * OT], F8,
                          kind="ExternalInput").ap()
    w2_d = nc.dram_tensor("w2", [OT, n_blk * H], F8,
                          kind="ExternalInput").ap()
    w3_d = nc.dram_tensor("w3", [H, CP], F16, kind="ExternalInput").ap()
    ones_d = nc.dram_tensor("ones", [GROUP * CP, GROUP * CP], F16,
                            kind="ExternalInput").ap()
    b2_d = nc.dram_tensor("b2", [H, 1], F32, kind="ExternalInput").ap()
    b3t_d = nc.dram_tensor("b3t", [GROUP * CP, 1], F32,
                           kind="ExternalInput").ap()
    out_d = nc.dram_tensor("out", [C, Bc], F32, kind="ExternalOutput").ap()

    with tile.TileContext(nc) as tc:
        with tc.tile_pool(name="wp", bufs=1) as wp, \
             tc.tile_pool(name="xsp", bufs=n_sup) as xsp, \
             tc.tile_pool(name="xbp", bufs=2) as xbp, \
             tc.tile_pool(name="y1p", bufs=15) as y1pp, \
             tc.tile_pool(name="y2p", bufs=4) as y2p, \
             tc.tile_pool(name="exp", bufs=2) as exp_p, \
             tc.tile_pool(name="y3p", bufs=2) as y3p, \
             tc.tile_pool(name="lsp", bufs=2) as lsp, \
             tc.tile_pool(name="op", bufs=2) as op, \
             tc.tile_pool(name="ps1", bufs=5, space="PSUM") as ps1, \
             tc.tile_pool(name="ps2", bufs=1, space="PSUM") as ps2, \
             tc.tile_pool(name="ps34", bufs=2, space="PSUM") as ps34:

            # ---- resident weights (SWDGE, ahead of the x slabs) ----
            # ws gates the very first matmul and the cold DMA path costs
            # ~0.18us per partition-row packet: split it across both HWDGE
            # rings so the halves transfer in parallel.
            ws_sb = wp.tile([PS, n_blk, 2, OT], F8)
            wsh = PS // 2
            nc.scalar.dma_start(ws_sb[:wsh], ws_d[:wsh, :])
            nc.sync.dma_start(ws_sb[wsh:], ws_d[wsh:, :])
            wb_sb = wp.tile([PB, n_blk, OT], F8)
            nc.sync.dma_start(wb_sb[:], wb_d[:])
            w2_sb = wp.tile([OT, n_blk, H], F8)
            nc.scalar.dma_start(w2_sb[:], w2_d[:])
            w3_sb = wp.tile([H, CP], F16)
            nc.sync.dma_start(w3_sb[:], w3_d[:])
            ones_sb = wp.tile([GROUP * CP, GROUP * CP], F16)
            nc.sync.dma_start(ones_sb[:], ones_d[:])
            b2_sb = wp.tile([H, 1], F32)
            nc.sync.dma_start(b2_sb[:], b2_d[:])
            b3t_sb = wp.tile([GROUP * CP, 1], F32)
            nc.sync.dma_start(b3t_sb[:], b3t_d[:])

            # ---- x slab loads: all xs first (small), then xb per slab.
            # Chunk-major layout: every per-chunk moving slice is fully
            # contiguous (the PE's fast path needs packed moving APs).
            n_half = SC // NB
            xs_tiles, xb_tiles = [], []
            for s in range(n_sup):
                xs_tiles.append(xsp.tile([PS, n_half, 2, NB], F8,
                                         name=f"xs{s}", tag="xs"))
                xb_tiles.append(xbp.tile([PB, n_half, n_blk, NB], F8,
                                         name=f"xb{s}", tag="xb"))
            # Slab 0 streams in per-chunk pieces so the first body's data
            # (~430KB) arrives long before the whole slab; later slabs load
            # whole, interleaved xs-then-xb (the cold DMA path runs at a
            # fraction of its steady rate, so first bytes matter most).
            xsw, xbw = 2 * NB, n_blk * NB
            for h in range(n_half):
                nc.gpsimd.dma_start(xs_tiles[0][:, h, :, :],
                                    xs_d[:, h * xsw:(h + 1) * xsw])
                nc.gpsimd.dma_start(xb_tiles[0][:, h, :, :],
                                    xb_d[:, h * xbw:(h + 1) * xbw])
            for s in range(1, n_sup):
                nc.gpsimd.dma_start(
                    xs_tiles[s][:], xs_d[:, s * 2 * SC:(s + 1) * 2 * SC])
                nc.gpsimd.dma_start(
                    xb_tiles[s][:], xb_d[:, s * n_blk * SC:(s + 1) * n_blk * SC])

            # Per-chunk state threaded through the software pipeline.
            y1_tiles = {}    # c -> [7 y1 tiles]
            p2_tiles = {}    # c -> L2 PSUM tile
            y2_tiles = {}    # c -> y2 SBUF tile
            p3_tiles = {}    # g -> L3 group PSUM tile
            ex_tiles = {}    # g -> exp SBUF tile
            y3_tiles = {}    # g -> logits+bias SBUF tile
            pending_tails = []  # (g, body_ready) awaiting softmax tail
            gsize = lambda g: min(GROUP, n_chunk - g * GROUP)

            def emit_l2_all(c):
                # The whole L2 for chunk c, deferred one body: every y1
                # eviction is long done, so the PE never waits here. The y2
                # eviction is NOT emitted here: it would head the epilogue
                # queue and block this body's L1 evictions behind the L2
                # chain. The body emits it after its second L1 eviction.
                p2_tiles[c] = ps2.tile([H, NB], F32, tag="l2", name="p2")
                for k in range(n_blk):
                    nc.tensor.matmul(p2_tiles[c][:], w2_sb[:, k, :],
                                     y1_tiles[c][k][:], start=(k == 0),
                                     stop=(k == n_blk - 1))
                del y1_tiles[c]
                emit_y2_evict(c)

            def emit_y2_evict(c):
                y2 = y2p.tile([H, NB], F16, tag="y2")
                if c % 2 == 0:
                    nc.scalar.activation(y2[:], p2_tiles[c][:], AF.Relu,
                                         bias=b2_sb[:, 0:1])
                else:
                    nc.vector.tensor_scalar(y2[:], p2_tiles[c][:],
                                            b2_sb[:, 0:1], 0.0,
                                            op0=ALU.add, op1=ALU.max)
                y2_tiles[c] = y2
                del p2_tiles[c]

            def emit_l3_group(g, body):
                # All of the group's col-32 L3 matmuls back to back: the PE
                # column-tile reconfig (128 -> 32 -> 128) is paid once per
                # group instead of once per chunk.
                gs = gsize(g)
                gp = gs * CP
                p3 = ps34.tile([GROUP * CP, NB], F32, tag="l34", name="p3")
                for m in range(gs):
                    c = g * GROUP + m
                    nc.tensor.matmul(p3[m * CP:(m + 1) * CP, :],
                                     w3_sb[:], y2_tiles[c][:],
                                     start=True, stop=True)
                    del y2_tiles[c]
                p3_tiles[g] = p3

            def emit_exp(g, body):
                # Exp + bias-add for a finished group — placed at body end so
                # the epilogue engines drain this body's evictions first.
                gs = gsize(g)
                gp = gs * CP
                p3 = p3_tiles.pop(g)
                ex = exp_p.tile([GROUP * CP, NB], F16, tag="ex")
                nc.scalar.activation(ex[:gp, :], p3[:gp, :],
                                     AF.Exp, bias=b3t_sb[:gp, 0:1])
                y3 = y3p.tile([GROUP * CP, NB], F32, tag="y3")
                nc.vector.tensor_scalar(y3[:gp, :], p3[:gp, :],
                                        b3t_sb[:gp, 0:1], None, op0=ALU.add)
                ex_tiles[g] = ex
                y3_tiles[g] = y3
                pending_tails.append((g, body))

            def emit_softmax_tail(g):
                gs = gsize(g)
                gp = gs * CP
                psl = ps34.tile([GROUP * CP, NB], F32, tag="l34", name="psl")
                nc.tensor.matmul(psl[:gp, :], ones_sb[:gp, :gp],
                                 ex_tiles[g][:gp, :], start=True, stop=True)
                ls = lsp.tile([GROUP * CP, NB], F32, tag="ls")
                nc.scalar.activation(ls[:gp, :], psl[:gp, :], AF.Ln)
                o = op.tile([GROUP * CP, NB], F32, tag="o")
                nc.vector.tensor_tensor(o[:gp, :], y3_tiles[g][:gp, :],
                                        ls[:gp, :], op=ALU.subtract)
                ring = nc.scalar if gs < GROUP else nc.sync
                for m in range(gs):
                    cc = g * GROUP + m
                    ring.dma_start(out_d[:, cc * NB:(cc + 1) * NB],
                                   o[m * CP:m * CP + C, :])
                del ex_tiles[g], y3_tiles[g]

            def flush_tails(body):
                while pending_tails and pending_tails[0][1] < body:
                    emit_softmax_tail(pending_tails.pop(0)[0])

            def l1_mms(dst_psum, j, xs_t, xb_t, h):
                # Stripe part as one fp8 DoubleRow matmul (both K-planes in
                # a single pass). With only ~1/3 of the stream in DR mode,
                # the chip holds full clock (the all-DR variant throttled).
                nc.tensor.matmul(dst_psum, ws_sb[:, j, :, :],
                                 xs_t[:, h, :, :],
                                 start=True, stop=False, perf_mode=DR)
                nc.tensor.matmul(dst_psum, wb_sb[:, j, :],
                                 xb_t[:, h, j, :],
                                 start=False, stop=True)

            for c in range(n_chunk):
                s, h = c // (SC // NB), c % (SC // NB)
                xs_t, xb_t = xs_tiles[s], xb_tiles[s]

                # Deferred work from earlier chunks heads the PE stream:
                # their data deps resolved most of a body ago, so no stalls.
                if c >= 1:
                    emit_l2_all(c - 1)

                # Seven L1 blocks, each three fp8 matmuls into one PSUM
                # bank, evicted by a pure ReLU+cast alternating ACT/DVE.
                y1s = []
                for j in range(n_blk):
                    p1 = ps1.tile([OT, NB], F32, tag="l1", name="p1")
                    l1_mms(p1[:], j, xs_t, xb_t, h)
                    y1 = y1pp.tile([OT, NB], F8, tag="y1")
                    if j % 2 == 0:
                        nc.scalar.activation(y1[:], p1[:], AF.Relu)
                    else:
                        nc.vector.tensor_scalar(y1[:], p1[:], 0.0, None,
                                                op0=ALU.max)
                    y1s.append(y1)
                y1_tiles[c] = y1s

                # Group tail work last: the epilogue engines finish this
                # body's evictions before touching exp/ln/subtract.
                if c >= 1 and ((c - 1) % GROUP == GROUP - 1
                               or c - 1 == n_chunk - 1):
                    g = (c - 1) // GROUP
                    emit_l3_group(g, c)
                    emit_exp(g, c)
                flush_tails(c)
                if c == n_chunk - 1:
                    # Start the final chunk's tail chain now: the PE waits
                    # briefly on this body's evictions, but that idle time
                    # would otherwise land in the drain anyway.
                    emit_l2_all(c)
                    g = c // GROUP
                    if c % GROUP != GROUP - 1:
                        emit_l3_group(g, c + 1)
                        emit_exp(g, c + 1)

            # ---- drain the pipeline ----
            for g, _ in list(pending_tails):
                emit_softmax_tail(g)
            pending_tails.clear()

    nc.compile()
    return nc


_CACHE = {}


def _prepare(x, W1, b1, W2, b2, W3, b3, mask1, mask2, mask3):
    B, D1 = x.shape
    H = W2.shape[0]
    C = W3.shape[0]
    assert B % N_CORES == 0
    Bc = B // N_CORES
    n_blk = D1 // OT
    n_pair = n_blk // 2
    n_sup = Bc // SC

    S, R_list = _decompose_mask1(np.asarray(mask1))
    nS = len(S)
    PS = (nS + 2 + 1) // 2              # stripe K-planes incl ones+zero rows
    maxR = max(len(r) for r in R_list)
    # >=65 partitions keeps the PE in its 128-row tile config: mixing 64-row
    # and 128-row matmuls in one stream costs a reconfig bubble per matmul.
    PB = max(maxR, 65)

    Wm1 = (np.asarray(W1) * np.asarray(mask1)).astype(np.float32)
    Wm2 = (np.asarray(W2) * np.asarray(mask2)).astype(np.float32)
    Wm3 = (np.asarray(W3) * np.asarray(mask3)).astype(np.float32)
    b1 = np.asarray(b1, np.float32)
    b2 = np.asarray(b2, np.float32)
    b3 = np.asarray(b3, np.float32)

    c8 = lambda a: np.asarray(a, dtype=NP8)
    c16 = lambda a: np.asarray(a, dtype=np.float16)

    # ---- stripe pack: K order = S cols, then ones row, then zero pad ----
    xT = np.asarray(x, np.float32).T                     # [D1, B]
    SP2 = 2 * PS
    xs_src = np.zeros((SP2, B), np.float32)
    xs_src[:nS] = xT[S]
    xs_src[nS] = 1.0
    # [NC, PS, n_sup, n_half, 2, NB] — chunk-major for contiguous slices
    n_half = SC // NB
    xs8 = (c8(xs_src).reshape(2, PS, N_CORES, n_sup, n_half, NB)
           .transpose(2, 1, 3, 4, 0, 5))
    xs8 = np.ascontiguousarray(xs8.reshape(N_CORES, PS, n_sup * 2 * SC))

    Ws_full = np.zeros((D1, SP2), np.float32)
    Ws_full[:, :nS] = Wm1[:, S]
    Ws_full[:, nS] = b1
    ws8 = np.ascontiguousarray(
        c8(Ws_full).reshape(n_blk, OT, 2, PS).transpose(3, 0, 2, 1)
        .reshape(PS, n_blk * 2 * OT))

    # ---- band pack (flat K, fp8 single matmuls) ----
    xb_src = np.zeros((n_blk, PB, B), np.float32)
    wb_src = np.zeros((n_blk, OT, PB), np.float32)
    for j, R in enumerate(R_list):
        xb_src[j, :len(R)] = xT[R]
        wb_src[j, :, :len(R)] = Wm1[j * OT:(j + 1) * OT, R]
    # [NC, PB, n_sup, n_half, n_blk, NB]
    xb8 = (c8(xb_src).reshape(n_blk, PB, N_CORES, n_sup, n_half, NB)
           .transpose(2, 1, 3, 4, 0, 5))
    xb8 = np.ascontiguousarray(
        xb8.reshape(N_CORES, PB, n_sup * n_blk * SC))
    wb8 = np.ascontiguousarray(
        c8(wb_src).transpose(2, 0, 1).reshape(PB, n_blk * OT))

    # ---- L2 pack: seven fp8 single matmuls ----
    t2 = Wm2.T.reshape(n_blk, OT, H)                     # [j, p, h]
    w2a8 = np.ascontiguousarray(
        c8(t2).transpose(1, 0, 2).reshape(OT, n_blk * H))

    # ---- L3 pack: classes padded to CP, 4 chunks per PE-tile group ----
    w3p = np.zeros((H, CP), np.float32)
    w3p[:, :C] = Wm3.T
    w3p16 = c16(w3p)
    GC = GROUP * CP
    ones = np.zeros((GC, GC), np.float32)
    for g in range(GROUP):
        ones[g * CP:g * CP + C, g * CP:g * CP + C] = 1.0
    ones16 = c16(ones)
    b3t = np.zeros((GC, 1), np.float32)
    for g in range(GROUP):
        b3t[g * CP:g * CP + C, 0] = b3
    b2p = b2.reshape(H, 1)

    meta = dict(PS=PS, PB=PB, Bc=Bc, D1=D1, H=H, C=C)
    key = (B, D1, H, C, nS, PB)
    if key not in _CACHE:
        _CACHE[key] = _build_program(meta)
    nc = _CACHE[key]

    in_maps = []
    for c in range(N_CORES):
        in_maps.append({
            "xs": xs8[c], "xb": xb8[c],
            "ws": ws8, "wb": wb8, "w2": w2a8,
            "w3": w3p16, "ones": ones16, "b2": b2p, "b3t": b3t,
        })
    return nc, in_maps, meta


def _assemble(results, meta):
    outs = [np.ascontiguousarray(results[c]["out"].T)     # [Bc, C]
            for c in range(N_CORES)]
    return np.concatenate(outs, axis=0).astype(np.float32)


def kernel(**inputs):
    nc, in_maps, meta = _prepare(**inputs)
    res = bass_utils.run_bass_kernel_spmd(nc, in_maps,
                                          core_ids=list(range(N_CORES)))
    return _assemble(res.results, meta)


def kernel_traced(tmpdir=None, **inputs):
    """Same as kernel() but with NTFF profiling; returns (output, results)."""
    nc, in_maps, meta = _prepare(**inputs)
    res = bass_utils.run_bass_kernel_spmd(nc, in_maps,
                                          core_ids=list(range(N_CORES)),
                                          trace=True, tmpdir=tmpdir)
    return _assemble(res.results, meta), res


# revision 45
# speedup vs baseline: 1.7853x; 1.0759x over previous
"""Trainium2 Bass kernel for the ButterflyMlp problem.

Computes log_softmax(L3(relu(L2(relu(L1(x)))))) where each Li is a masked
linear layer (butterfly sparsity: global column stripes + a diagonal band),
batch 65536, data-parallel over 8 NeuronCores (8192 rows/core).

Strategy (per core, feature-major, batch chunks of 512 columns):
  - L1 exploits butterfly structure: stripe columns S (204, dense for every
    output row) are a shared K-axis split in two 103-row planes; each
    112-row output block adds a narrow band residual (<=92 cols). All
    matmuls are fp8e4 single-row mode: on this platform, 8 cores running
    fp8 DoubleRow trigger a chip-level clock throttle (~1.4GHz) that
    exactly cancels DoubleRow's 2x K-throughput, while single-row fp8/fp16
    streams sustain the full 2.4GHz (1 moving column/cycle).
  - Every matmul keeps >=65 K-partitions so the PE stays in its 128-row
    tile config; mixing 64-row and 128-row tiles costs a reconfig bubble
    per matmul (~1.7x slowdown measured).
  - L1 bias is folded into the stripe matmul via an appended ones-row in
    the packed x (weight row = b1), so PSUM evictions are pure ReLU+cast
    ops alternating ScalarE/DVE (Pool cannot read PSUM).
  - Each chunk's L2 (7 fp8 matmuls, K=112) is deferred one chunk so its y1
    evictions are long done when the PE reaches it; b2 is applied by the
    y2 eviction (ACT bias / DVE tensor_scalar, alternating parity).
  - L3 (K=128, fp16) pads classes 10->32; three consecutive chunks write
    one PSUM bank at partition offsets 0/32/64 (PE column tiling, emitted
    back to back to amortize the col-32 reconfig), so exp/ln/subtract of
    log_softmax run once per 3 chunks and logsumexp is a single
    ones-blockdiag fp16 matmul. exp/+bias read PSUM directly; the final
    subtract runs on DVE.
  - x is pre-gathered and fp8-packed on host in chunk-major layout so all
    moving APs are contiguous. The cold DMA path costs ~0.18us per
    partition-row packet, so startup is packet-count-bound: ws splits
    across both HWDGE rings, slab 0 streams in per-chunk pieces on SWDGE,
    and the weights ride the rings in parallel with the x stream. Outputs
    go back on the sync HWDGE ring (scalar ring for the final ragged
    group), 3 chunks per transfer.
"""
import sys
sys.path.insert(0, "/opt/trn_rl_repo")
import numpy as np
import ml_dtypes

import concourse.bass as bass
import concourse.bacc as bacc
import concourse.mybir as mybir
import concourse.tile as tile
from concourse import bass_utils

F32 = mybir.dt.float32
F16 = mybir.dt.float16
F8 = mybir.dt.float8e4
NP8 = ml_dtypes.float8_e4m3
AF = mybir.ActivationFunctionType
ALU = mybir.AluOpType
DR = mybir.MatmulPerfMode.DoubleRow

# Keep every ACT function this kernel uses (Relu/Exp/Ln + implicit Copy /
# Identity) inside one activation-table set so the greedy per-function set
# chooser emits a single table load instead of reloading per chunk.
_PIN_SET = "natural_log_exp_and_others"
_orig_gat = bacc.get_activation_tables


def _pinned_gat(arch):
    tabs = _orig_gat(arch)
    need = {AF.Relu, AF.Identity, AF.Exp, AF.Ln, AF.Copy}
    if _PIN_SET in tabs and need <= tabs[_PIN_SET]:
        for name in tabs:
            if name != _PIN_SET:
                tabs[name] = tabs[name] - need
    return tabs


bacc.get_activation_tables = _pinned_gat

N_CORES = 8
NB = 512          # batch columns per matmul (one fp32 PSUM bank)
SC = 2048         # batch columns per DMA slab
OT = 112          # L1 output block width (784/7)
GROUP = 3         # batch chunks per L3/log-softmax group (3*32 = 96 rows;
                  # AP base partitions are limited to 0/32/64)
CP = 32           # padded class count (PE tile col granularity)


def _decompose_mask1(mask1):
    D_out, D_in = mask1.shape
    S = np.where(mask1.all(axis=0))[0]
    n_blk = (D_out + OT - 1) // OT
    stripe_set = np.zeros(D_in, dtype=bool)
    stripe_set[S] = True
    R_list = []
    for j in range(n_blk):
        blk = mask1[j * OT:(j + 1) * OT]
        R_list.append(np.where(blk.any(axis=0) & ~stripe_set)[0])
    return S, R_list


def _build_program(meta):
    PS, PB = meta["PS"], meta["PB"]
    Bc = meta["Bc"]
    D1, H, C = meta["D1"], meta["H"], meta["C"]
    n_blk = D1 // OT
    n_pair = n_blk // 2                 # L2 DoubleRow pairs (tail is fp16)
    n_sup = Bc // SC
    n_chunk = Bc // NB

    nc = bacc.Bacc("TRN2", target_bir_lowering=False, debug=False,
                   enable_asserts=False, num_devices=N_CORES)

    xs_d = nc.dram_tensor("xs", [PS, n_sup * 2 * SC], F8,
                          kind="ExternalInput").ap()
    xb_d = nc.dram_tensor("xb", [PB, n_sup * n_blk * SC], F8,
                          kind="ExternalInput").ap()
    ws_d = nc.dram_tensor("ws", [PS, n_blk * 2 * OT], F8,
                          kind="ExternalInput").ap()
    wb_d = nc.dram_tensor("wb", [PB, n_blk * OT], F8,
                          kind="ExternalInput").ap()
    w2_d = nc.dram_tensor("w2", [OT, n_blk * H], F8,
                          kind="ExternalInput").ap()
    w3_d = nc.dram_tensor("w3", [H, CP], F16, kind="ExternalInput").ap()
    ones_d = nc.dram_tensor("ones", [GROUP * CP, GROUP * CP], F16,
                            kind="ExternalInput").ap()
    b2_d = nc.dram_tensor("b2", [H, 1], F32, kind="ExternalInput").ap()
    b3t_d = nc.dram_tensor("b3t", [GROUP * CP, 1], F32,
                           kind="ExternalInput").ap()
    out_d = nc.dram_tensor("out", [C, Bc], F32, kind="ExternalOutput").ap()

    with tile.TileContext(nc) as tc:
        with tc.tile_pool(name="wp", bufs=1) as wp, \
             tc.tile_pool(name="xsp", bufs=n_sup) as xsp, \
             tc.tile_pool(name="xbp", bufs=2) as xbp, \
             tc.tile_pool(name="y1p", bufs=8) as y1pp, \
             tc.tile_pool(name="y1s", bufs=3) as y1sp, \
             tc.tile_pool(name="y2p", bufs=4) as y2p, \
             tc.tile_pool(name="exp", bufs=2) as exp_p, \
             tc.tile_pool(name="y3p", bufs=2) as y3p, \
             tc.tile_pool(name="lsp", bufs=2) as lsp, \
             tc.tile_pool(name="op", bufs=2) as op, \
             tc.tile_pool(name="ps1", bufs=5, space="PSUM") as ps1, \
             tc.tile_pool(name="ps2", bufs=1, space="PSUM") as ps2, \
             tc.tile_pool(name="ps34", bufs=2, space="PSUM") as ps34:

            # ---- resident weights (SWDGE, ahead of the x slabs) ----
            # ws gates the very first matmul and the cold DMA path costs
            # ~0.18us per partition-row packet: split it across both HWDGE
            # rings so the halves transfer in parallel.
            ws_sb = wp.tile([PS, n_blk, 2, OT], F8)
            wsh = PS // 2
            nc.scalar.dma_start(ws_sb[:wsh], ws_d[:wsh, :])
            nc.sync.dma_start(ws_sb[wsh:], ws_d[wsh:, :])
            wb_sb = wp.tile([PB, n_blk, OT], F8)
            nc.sync.dma_start(wb_sb[:], wb_d[:])
            w2_sb = wp.tile([OT, n_blk, H], F8)
            nc.scalar.dma_start(w2_sb[:], w2_d[:])
            w3_sb = wp.tile([H, CP], F16)
            nc.sync.dma_start(w3_sb[:], w3_d[:])
            ones_sb = wp.tile([GROUP * CP, GROUP * CP], F16)
            nc.sync.dma_start(ones_sb[:], ones_d[:])
            b2_sb = wp.tile([H, 1], F32)
            nc.sync.dma_start(b2_sb[:], b2_d[:])
            b3t_sb = wp.tile([GROUP * CP, 1], F32)
            nc.sync.dma_start(b3t_sb[:], b3t_d[:])

            # ---- x slab loads: all xs first (small), then xb per slab.
            # Chunk-major layout: every per-chunk moving slice is fully
            # contiguous (the PE's fast path needs packed moving APs).
            n_half = SC // NB
            xs_tiles, xb_tiles = [], []
            for s in range(n_sup):
                xs_tiles.append(xsp.tile([PS, n_half, 2, NB], F8,
                                         name=f"xs{s}", tag="xs"))
                xb_tiles.append(xbp.tile([PB, n_half, n_blk, NB], F8,
                                         name=f"xb{s}", tag="xb"))
            # Slab 0 streams in per-chunk pieces so the first body's data
            # (~430KB) arrives long before the whole slab; later slabs load
            # whole, interleaved xs-then-xb (the cold DMA path runs at a
            # fraction of its steady rate, so first bytes matter most).
            xsw, xbw = 2 * NB, n_blk * NB
            for h in range(n_half):
                nc.gpsimd.dma_start(xs_tiles[0][:, h, :, :],
                                    xs_d[:, h * xsw:(h + 1) * xsw])
                nc.gpsimd.dma_start(xb_tiles[0][:, h, :, :],
                                    xb_d[:, h * xbw:(h + 1) * xbw])
            for s in range(1, n_sup):
                nc.gpsimd.dma_start(
                    xs_tiles[s][:], xs_d[:, s * 2 * SC:(s + 1) * 2 * SC])
                nc.gpsimd.dma_start(
                    xb_tiles[s][:], xb_d[:, s * n_blk * SC:(s + 1) * n_blk * SC])

            # Per-chunk state threaded through the software pipeline.
            y1_tiles = {}    # c -> [7 y1 tiles]
            p2_tiles = {}    # c -> L2 PSUM tile
            y2_tiles = {}    # c -> y2 SBUF tile
            p3_tiles = {}    # g -> L3 group PSUM tile
            ex_tiles = {}    # g -> exp SBUF tile
            y3_tiles = {}    # g -> logits+bias SBUF tile
            pending_tails = []  # (g, body_ready) awaiting softmax tail
            gsize = lambda g: min(GROUP, n_chunk - g * GROUP)

            def emit_l2_all(c):
                # The whole L2 for chunk c, deferred one body: every y1
                # eviction is long done, so the PE never waits here. The y2
                # eviction is NOT emitted here: it would head the epilogue
                # queue and block this body's L1 evictions behind the L2
                # chain. The body emits it after its second L1 eviction.
                p2_tiles[c] = ps2.tile([H, NB], F32, tag="l2", name="p2")
                for k in range(n_blk // 2):
                    nc.tensor.matmul(p2_tiles[c][:], w2_sb[:, 2 * k:2 * k + 2, :],
                                     y1_tiles[c][k][:], start=(k == 0),
                                     stop=False, perf_mode=DR)
                nc.tensor.matmul(p2_tiles[c][:], w2_sb[:, n_blk - 1, :],
                                 y1_tiles[c][3][:], start=False, stop=True)
                del y1_tiles[c]
                emit_y2_evict(c)

            def emit_y2_evict(c):
                y2 = y2p.tile([H, NB], F16, tag="y2")
                if c % 2 == 0:
                    nc.scalar.activation(y2[:], p2_tiles[c][:], AF.Relu,
                                         bias=b2_sb[:, 0:1])
                else:
                    nc.vector.tensor_scalar(y2[:], p2_tiles[c][:],
                                            b2_sb[:, 0:1], 0.0,
                                            op0=ALU.add, op1=ALU.max)
                y2_tiles[c] = y2
                del p2_tiles[c]

            def emit_l3_group(g, body):
                # All of the group's col-32 L3 matmuls back to back: the PE
                # column-tile reconfig (128 -> 32 -> 128) is paid once per
                # group instead of once per chunk.
                gs = gsize(g)
                gp = gs * CP
                p3 = ps34.tile([GROUP * CP, NB], F32, tag="l34", name="p3")
                for m in range(gs):
                    c = g * GROUP + m
                    nc.tensor.matmul(p3[m * CP:(m + 1) * CP, :],
                                     w3_sb[:], y2_tiles[c][:],
                                     start=True, stop=True)
                    del y2_tiles[c]
                p3_tiles[g] = p3

            def emit_exp(g, body):
                # Exp + bias-add for a finished group — placed at body end so
                # the epilogue engines drain this body's evictions first.
                gs = gsize(g)
                gp = gs * CP
                p3 = p3_tiles.pop(g)
                ex = exp_p.tile([GROUP * CP, NB], F16, tag="ex")
                nc.scalar.activation(ex[:gp, :], p3[:gp, :],
                                     AF.Exp, bias=b3t_sb[:gp, 0:1])
                y3 = y3p.tile([GROUP * CP, NB], F32, tag="y3")
                nc.vector.tensor_scalar(y3[:gp, :], p3[:gp, :],
                                        b3t_sb[:gp, 0:1], None, op0=ALU.add)
                ex_tiles[g] = ex
                y3_tiles[g] = y3
                pending_tails.append((g, body))

            def emit_softmax_tail(g):
                gs = gsize(g)
                gp = gs * CP
                psl = ps34.tile([GROUP * CP, NB], F32, tag="l34", name="psl")
                nc.tensor.matmul(psl[:gp, :], ones_sb[:gp, :gp],
                                 ex_tiles[g][:gp, :], start=True, stop=True)
                ls = lsp.tile([GROUP * CP, NB], F32, tag="ls")
                nc.scalar.activation(ls[:gp, :], psl[:gp, :], AF.Ln)
                o = op.tile([GROUP * CP, NB], F32, tag="o")
                nc.vector.tensor_tensor(o[:gp, :], y3_tiles[g][:gp, :],
                                        ls[:gp, :], op=ALU.subtract)
                ring = nc.scalar if gs < GROUP else nc.sync
                for m in range(gs):
                    cc = g * GROUP + m
                    ring.dma_start(out_d[:, cc * NB:(cc + 1) * NB],
                                   o[m * CP:m * CP + C, :])
                del ex_tiles[g], y3_tiles[g]

            def flush_tails(body):
                while pending_tails and pending_tails[0][1] < body:
                    emit_softmax_tail(pending_tails.pop(0)[0])

            def l1_mms(dst_psum, j, xs_t, xb_t, h):
                # Stripe part as one fp8 DoubleRow matmul (both K-planes in
                # a single pass). With only ~1/3 of the stream in DR mode,
                # the chip holds full clock (the all-DR variant throttled).
                nc.tensor.matmul(dst_psum, ws_sb[:, j, :, :],
                                 xs_t[:, h, :, :],
                                 start=True, stop=False, perf_mode=DR)
                nc.tensor.matmul(dst_psum, wb_sb[:, j, :],
                                 xb_t[:, h, j, :],
                                 start=False, stop=True)

            for c in range(n_chunk):
                s, h = c // (SC // NB), c % (SC // NB)
                xs_t, xb_t = xs_tiles[s], xb_tiles[s]

                # Deferred work from earlier chunks heads the PE stream:
                # their data deps resolved most of a body ago, so no stalls.
                if c >= 1:
                    emit_l2_all(c - 1)

                # Seven L1 blocks, each three fp8 matmuls into one PSUM
                # bank, evicted by a pure ReLU+cast alternating ACT/DVE.
                # y1 lands in DoubleRow pair tiles [OT, 2, NB] (blocks
                # 2k/2k+1 in the two K-planes); block 6 gets a flat tile
                # for the fp8 single tail matmul.
                y1s = []
                for j in range(n_blk):
                    p1 = ps1.tile([OT, NB], F32, tag="l1", name="p1")
                    l1_mms(p1[:], j, xs_t, xb_t, h)
                    if j < n_blk - 1:
                        if j % 2 == 0:
                            y1s.append(y1pp.tile([OT, 2, NB], F8, tag="y1", name="y1"))
                        dst = y1s[j // 2][:, j % 2, :]
                    else:
                        y1s.append(y1sp.tile([OT, NB], F8, tag="y1s", name="y1t"))
                        dst = y1s[3][:]
                    if j % 2 == 0:
                        nc.scalar.activation(dst, p1[:], AF.Relu)
                    else:
                        nc.vector.tensor_scalar(dst, p1[:], 0.0, None,
                                                op0=ALU.max)
                y1_tiles[c] = y1s

                # Group tail work last: the epilogue engines finish this
                # body's evictions before touching exp/ln/subtract.
                if c >= 1 and ((c - 1) % GROUP == GROUP - 1
                               or c - 1 == n_chunk - 1):
                    g = (c - 1) // GROUP
                    emit_l3_group(g, c)
                    emit_exp(g, c)
                flush_tails(c)
                if c == n_chunk - 1:
                    # Start the final chunk's tail chain now: the PE waits
                    # briefly on this body's evictions, but that idle time
                    # would otherwise land in the drain anyway.
                    emit_l2_all(c)
                    g = c // GROUP
                    if c % GROUP != GROUP - 1:
                        emit_l3_group(g, c + 1)
                        emit_exp(g, c + 1)

            # ---- drain the pipeline ----
            for g, _ in list(pending_tails):
                emit_softmax_tail(g)
            pending_tails.clear()

    nc.compile()
    return nc


_CACHE = {}


def _prepare(x, W1, b1, W2, b2, W3, b3, mask1, mask2, mask3):
    B, D1 = x.shape
    H = W2.shape[0]
    C = W3.shape[0]
    assert B % N_CORES == 0
    Bc = B // N_CORES
    n_blk = D1 // OT
    n_pair = n_blk // 2
    n_sup = Bc // SC

    S, R_list = _decompose_mask1(np.asarray(mask1))
    nS = len(S)
    PS = (nS + 2 + 1) // 2              # stripe K-planes incl ones+zero rows
    maxR = max(len(r) for r in R_list)
    # >=65 partitions keeps the PE in its 128-row tile config: mixing 64-row
    # and 128-row matmuls in one stream costs a reconfig bubble per matmul.
    PB = max(maxR, 65)

    Wm1 = (np.asarray(W1) * np.asarray(mask1)).astype(np.float32)
    Wm2 = (np.asarray(W2) * np.asarray(mask2)).astype(np.float32)
    Wm3 = (np.asarray(W3) * np.asarray(mask3)).astype(np.float32)
    b1 = np.asarray(b1, np.float32)
    b2 = np.asarray(b2, np.float32)
    b3 = np.asarray(b3, np.float32)

    c8 = lambda a: np.asarray(a, dtype=NP8)
    c16 = lambda a: np.asarray(a, dtype=np.float16)

    # ---- stripe pack: K order = S cols, then ones row, then zero pad ----
    xT = np.asarray(x, np.float32).T                     # [D1, B]
    SP2 = 2 * PS
    xs_src = np.zeros((SP2, B), np.float32)
    xs_src[:nS] = xT[S]
    xs_src[nS] = 1.0
    # [NC, PS, n_sup, n_half, 2, NB] — chunk-major for contiguous slices
    n_half = SC // NB
    xs8 = (c8(xs_src).reshape(2, PS, N_CORES, n_sup, n_half, NB)
           .transpose(2, 1, 3, 4, 0, 5))
    xs8 = np.ascontiguousarray(xs8.reshape(N_CORES, PS, n_sup * 2 * SC))

    Ws_full = np.zeros((D1, SP2), np.float32)
    Ws_full[:, :nS] = Wm1[:, S]
    Ws_full[:, nS] = b1
    ws8 = np.ascontiguousarray(
        c8(Ws_full).reshape(n_blk, OT, 2, PS).transpose(3, 0, 2, 1)
        .reshape(PS, n_blk * 2 * OT))

    # ---- band pack (flat K, fp8 single matmuls) ----
    xb_src = np.zeros((n_blk, PB, B), np.float32)
    wb_src = np.zeros((n_blk, OT, PB), np.float32)
    for j, R in enumerate(R_list):
        xb_src[j, :len(R)] = xT[R]
        wb_src[j, :, :len(R)] = Wm1[j * OT:(j + 1) * OT, R]
    # [NC, PB, n_sup, n_half, n_blk, NB]
    xb8 = (c8(xb_src).reshape(n_blk, PB, N_CORES, n_sup, n_half, NB)
           .transpose(2, 1, 3, 4, 0, 5))
    xb8 = np.ascontiguousarray(
        xb8.reshape(N_CORES, PB, n_sup * n_blk * SC))
    wb8 = np.ascontiguousarray(
        c8(wb_src).transpose(2, 0, 1).reshape(PB, n_blk * OT))

    # ---- L2 pack: seven fp8 single matmuls ----
    t2 = Wm2.T.reshape(n_blk, OT, H)                     # [j, p, h]
    w2a8 = np.ascontiguousarray(
        c8(t2).transpose(1, 0, 2).reshape(OT, n_blk * H))

    # ---- L3 pack: classes padded to CP, 4 chunks per PE-tile group ----
    w3p = np.zeros((H, CP), np.float32)
    w3p[:, :C] = Wm3.T
    w3p16 = c16(w3p)
    GC = GROUP * CP
    ones = np.zeros((GC, GC), np.float32)
    for g in range(GROUP):
        ones[g * CP:g * CP + C, g * CP:g * CP + C] = 1.0
    ones16 = c16(ones)
    b3t = np.zeros((GC, 1), np.float32)
    for g in range(GROUP):
        b3t[g * CP:g * CP + C, 0] = b3
    b2p = b2.reshape(H, 1)

    meta = dict(PS=PS, PB=PB, Bc=Bc, D1=D1, H=H, C=C)
    key = (B, D1, H, C, nS, PB)
    if key not in _CACHE:
        _CACHE[key] = _build_program(meta)
    nc = _CACHE[key]

    in_maps = []
    for c in range(N_CORES):
        in_maps.append({
            "xs": xs8[c], "xb": xb8[c],
            "ws": ws8, "wb": wb8, "w2": w2a8,
            "w3": w3p16, "ones": ones16, "b2": b2p, "b3t": b3t,
        })
    return nc, in_maps, meta


def _assemble(results, meta):
    outs = [np.ascontiguousarray(results[c]["out"].T)     # [Bc, C]
            for c in range(N_CORES)]
    return np.concatenate(outs, axis=0).astype(np.float32)


def kernel(**inputs):
    nc, in_maps, meta = _prepare(**inputs)
    res = bass_utils.run_bass_kernel_spmd(nc, in_maps,
                                          core_ids=list(range(N_CORES)))
    return _assemble(res.results, meta)


def kernel_traced(tmpdir=None, **inputs):
    """Same as kernel() but with NTFF profiling; returns (output, results)."""
    nc, in_maps, meta = _prepare(**inputs)
    res = bass_utils.run_bass_kernel_spmd(nc, in_maps,
                                          core_ids=list(range(N_CORES)),
                                          trace=True, tmpdir=tmpdir)
    return _assemble(res.results, meta), res


# revision 46
# speedup vs baseline: 1.7939x; 1.0048x over previous
"""Trainium2 Bass kernel for the ButterflyMlp problem.

Computes log_softmax(L3(relu(L2(relu(L1(x)))))) where each Li is a masked
linear layer (butterfly sparsity: global column stripes + a diagonal band),
batch 65536, data-parallel over 8 NeuronCores (8192 rows/core).

Strategy (per core, feature-major, batch chunks of 512 columns):
  - L1 exploits butterfly structure: stripe columns S (204, dense for every
    output row) are a shared K-axis split in two 103-row planes; each
    112-row output block adds a narrow band residual (<=92 cols). All
    matmuls are fp8e4 single-row mode: on this platform, 8 cores running
    fp8 DoubleRow trigger a chip-level clock throttle (~1.4GHz) that
    exactly cancels DoubleRow's 2x K-throughput, while single-row fp8/fp16
    streams sustain the full 2.4GHz (1 moving column/cycle).
  - Every matmul keeps >=65 K-partitions so the PE stays in its 128-row
    tile config; mixing 64-row and 128-row tiles costs a reconfig bubble
    per matmul (~1.7x slowdown measured).
  - L1 bias is folded into the stripe matmul via an appended ones-row in
    the packed x (weight row = b1), so PSUM evictions are pure ReLU+cast
    ops alternating ScalarE/DVE (Pool cannot read PSUM).
  - Each chunk's L2 (7 fp8 matmuls, K=112) is deferred one chunk so its y1
    evictions are long done when the PE reaches it; b2 is applied by the
    y2 eviction (ACT bias / DVE tensor_scalar, alternating parity).
  - L3 (K=128, fp16) pads classes 10->32; three consecutive chunks write
    one PSUM bank at partition offsets 0/32/64 (PE column tiling, emitted
    back to back to amortize the col-32 reconfig), so exp/ln/subtract of
    log_softmax run once per 3 chunks and logsumexp is a single
    ones-blockdiag fp16 matmul. exp/+bias read PSUM directly; the final
    subtract runs on DVE.
  - x is pre-gathered and fp8-packed on host in chunk-major layout so all
    moving APs are contiguous. The cold DMA path costs ~0.18us per
    partition-row packet, so startup is packet-count-bound: ws splits
    across both HWDGE rings, slab 0 streams in per-chunk pieces on SWDGE,
    and the weights ride the rings in parallel with the x stream. Outputs
    go back on the sync HWDGE ring (scalar ring for the final ragged
    group), 3 chunks per transfer.
"""
import sys
sys.path.insert(0, "/opt/trn_rl_repo")
import numpy as np
import ml_dtypes

import concourse.bass as bass
import concourse.bacc as bacc
import concourse.mybir as mybir
import concourse.tile as tile
from concourse import bass_utils

F32 = mybir.dt.float32
F16 = mybir.dt.float16
F8 = mybir.dt.float8e4
NP8 = ml_dtypes.float8_e4m3
AF = mybir.ActivationFunctionType
ALU = mybir.AluOpType
DR = mybir.MatmulPerfMode.DoubleRow

# Keep every ACT function this kernel uses (Relu/Exp/Ln + implicit Copy /
# Identity) inside one activation-table set so the greedy per-function set
# chooser emits a single table load instead of reloading per chunk.
_PIN_SET = "natural_log_exp_and_others"
_orig_gat = bacc.get_activation_tables


def _pinned_gat(arch):
    tabs = _orig_gat(arch)
    need = {AF.Relu, AF.Identity, AF.Exp, AF.Ln, AF.Copy}
    if _PIN_SET in tabs and need <= tabs[_PIN_SET]:
        for name in tabs:
            if name != _PIN_SET:
                tabs[name] = tabs[name] - need
    return tabs


bacc.get_activation_tables = _pinned_gat

N_CORES = 8
NB = 512          # batch columns per matmul (one fp32 PSUM bank)
SC = 2048         # batch columns per DMA slab
OT = 112          # L1 output block width (784/7)
GROUP = 3         # batch chunks per L3/log-softmax group (3*32 = 96 rows;
                  # AP base partitions are limited to 0/32/64)
CP = 32           # padded class count (PE tile col granularity)


def _decompose_mask1(mask1):
    D_out, D_in = mask1.shape
    S = np.where(mask1.all(axis=0))[0]
    n_blk = (D_out + OT - 1) // OT
    stripe_set = np.zeros(D_in, dtype=bool)
    stripe_set[S] = True
    R_list = []
    for j in range(n_blk):
        blk = mask1[j * OT:(j + 1) * OT]
        R_list.append(np.where(blk.any(axis=0) & ~stripe_set)[0])
    return S, R_list


def _build_program(meta):
    PS, PB = meta["PS"], meta["PB"]
    Bc = meta["Bc"]
    D1, H, C = meta["D1"], meta["H"], meta["C"]
    n_blk = D1 // OT
    n_pair = n_blk // 2                 # L2 DoubleRow pairs (tail is fp16)
    n_sup = Bc // SC
    n_chunk = Bc // NB

    nc = bacc.Bacc("TRN2", target_bir_lowering=False, debug=False,
                   enable_asserts=False, num_devices=N_CORES)

    xs_d = nc.dram_tensor("xs", [PS, n_sup * 2 * SC], F8,
                          kind="ExternalInput").ap()
    xb_d = nc.dram_tensor("xb", [PB, n_sup * n_blk * SC], F8,
                          kind="ExternalInput").ap()
    ws_d = nc.dram_tensor("ws", [PS, n_blk * 2 * OT], F8,
                          kind="ExternalInput").ap()
    wb_d = nc.dram_tensor("wb", [PB, n_blk * OT], F8,
                          kind="ExternalInput").ap()
    w2_d = nc.dram_tensor("w2", [OT, n_blk * H], F8,
                          kind="ExternalInput").ap()
    w3_d = nc.dram_tensor("w3", [H, CP], F16, kind="ExternalInput").ap()
    ones_d = nc.dram_tensor("ones", [GROUP * CP, GROUP * CP], F16,
                            kind="ExternalInput").ap()
    b2_d = nc.dram_tensor("b2", [H, 1], F32, kind="ExternalInput").ap()
    b3t_d = nc.dram_tensor("b3t", [GROUP * CP, 1], F32,
                           kind="ExternalInput").ap()
    out_d = nc.dram_tensor("out", [C, Bc], F32, kind="ExternalOutput").ap()

    with tile.TileContext(nc) as tc:
        with tc.tile_pool(name="wp", bufs=1) as wp, \
             tc.tile_pool(name="xsp", bufs=n_sup) as xsp, \
             tc.tile_pool(name="xbp", bufs=2) as xbp, \
             tc.tile_pool(name="y1p", bufs=8) as y1pp, \
             tc.tile_pool(name="y1s", bufs=3) as y1sp, \
             tc.tile_pool(name="y2p", bufs=4) as y2p, \
             tc.tile_pool(name="exp", bufs=2) as exp_p, \
             tc.tile_pool(name="y3p", bufs=2) as y3p, \
             tc.tile_pool(name="lsp", bufs=2) as lsp, \
             tc.tile_pool(name="op", bufs=2) as op, \
             tc.tile_pool(name="ps1", bufs=5, space="PSUM") as ps1, \
             tc.tile_pool(name="ps2", bufs=1, space="PSUM") as ps2, \
             tc.tile_pool(name="ps34", bufs=2, space="PSUM") as ps34:

            # ---- resident weights (SWDGE, ahead of the x slabs) ----
            # ws gates the very first matmul and the cold DMA path costs
            # ~0.18us per partition-row packet: split it across both HWDGE
            # rings so the halves transfer in parallel.
            ws_sb = wp.tile([PS, n_blk, 2, OT], F8)
            wsh = PS // 2
            nc.scalar.dma_start(ws_sb[:wsh], ws_d[:wsh, :])
            nc.sync.dma_start(ws_sb[wsh:], ws_d[wsh:, :])
            wb_sb = wp.tile([PB, n_blk, OT], F8)
            nc.sync.dma_start(wb_sb[:], wb_d[:])
            w2_sb = wp.tile([OT, n_blk, H], F8)
            nc.scalar.dma_start(w2_sb[:], w2_d[:])
            w3_sb = wp.tile([H, CP], F16)
            nc.sync.dma_start(w3_sb[:], w3_d[:])
            ones_sb = wp.tile([GROUP * CP, GROUP * CP], F16)
            nc.sync.dma_start(ones_sb[:], ones_d[:])
            b2_sb = wp.tile([H, 1], F32)
            nc.sync.dma_start(b2_sb[:], b2_d[:])
            b3t_sb = wp.tile([GROUP * CP, 1], F32)
            nc.sync.dma_start(b3t_sb[:], b3t_d[:])

            # ---- x slab loads: all xs first (small), then xb per slab.
            # Chunk-major layout: every per-chunk moving slice is fully
            # contiguous (the PE's fast path needs packed moving APs).
            n_half = SC // NB
            xs_tiles, xb_tiles = [], []
            for s in range(n_sup):
                xs_tiles.append(xsp.tile([PS, n_half, 2, NB], F8,
                                         name=f"xs{s}", tag="xs"))
                xb_tiles.append(xbp.tile([PB, n_half, n_blk, NB], F8,
                                         name=f"xb{s}", tag="xb"))
            # Slab 0 streams in per-chunk pieces so the first body's data
            # (~430KB) arrives long before the whole slab; later slabs load
            # whole, interleaved xs-then-xb (the cold DMA path runs at a
            # fraction of its steady rate, so first bytes matter most).
            xsw, xbw = 2 * NB, n_blk * NB
            for h in range(n_half):
                nc.gpsimd.dma_start(xs_tiles[0][:, h, :, :],
                                    xs_d[:, h * xsw:(h + 1) * xsw])
                nc.gpsimd.dma_start(xb_tiles[0][:, h, :, :],
                                    xb_d[:, h * xbw:(h + 1) * xbw])
            for s in range(1, n_sup):
                nc.gpsimd.dma_start(
                    xs_tiles[s][:], xs_d[:, s * 2 * SC:(s + 1) * 2 * SC])
                nc.gpsimd.dma_start(
                    xb_tiles[s][:], xb_d[:, s * n_blk * SC:(s + 1) * n_blk * SC])

            # Per-chunk state threaded through the software pipeline.
            y1_tiles = {}    # c -> [7 y1 tiles]
            p2_tiles = {}    # c -> L2 PSUM tile
            y2_tiles = {}    # c -> y2 SBUF tile
            p3_tiles = {}    # g -> L3 group PSUM tile
            ex_tiles = {}    # g -> exp SBUF tile
            y3_tiles = {}    # g -> logits+bias SBUF tile
            pending_tails = []  # (g, body_ready) awaiting softmax tail
            gsize = lambda g: min(GROUP, n_chunk - g * GROUP)

            def emit_l2_all(c):
                # The whole L2 for chunk c, deferred one body: every y1
                # eviction is long done, so the PE never waits here. The y2
                # eviction is NOT emitted here: it would head the epilogue
                # queue and block this body's L1 evictions behind the L2
                # chain. The body emits it after its second L1 eviction.
                p2_tiles[c] = ps2.tile([H, NB], F32, tag="l2", name="p2")
                for k in range(n_blk // 2):
                    nc.tensor.matmul(p2_tiles[c][:], w2_sb[:, 2 * k:2 * k + 2, :],
                                     y1_tiles[c][k][:], start=(k == 0),
                                     stop=False, perf_mode=DR)
                nc.tensor.matmul(p2_tiles[c][:], w2_sb[:, n_blk - 1, :],
                                 y1_tiles[c][3][:], start=False, stop=True)
                del y1_tiles[c]
                emit_y2_evict(c)

            def emit_y2_evict(c):
                y2 = y2p.tile([H, NB], F16, tag="y2")
                nc.scalar.activation(y2[:], p2_tiles[c][:], AF.Relu,
                                     bias=b2_sb[:, 0:1])
                y2_tiles[c] = y2
                del p2_tiles[c]

            def emit_l3_group(g, body):
                # All of the group's col-32 L3 matmuls back to back: the PE
                # column-tile reconfig (128 -> 32 -> 128) is paid once per
                # group instead of once per chunk.
                gs = gsize(g)
                gp = gs * CP
                p3 = ps34.tile([GROUP * CP, NB], F32, tag="l34", name="p3")
                for m in range(gs):
                    c = g * GROUP + m
                    nc.tensor.matmul(p3[m * CP:(m + 1) * CP, :],
                                     w3_sb[:], y2_tiles[c][:],
                                     start=True, stop=True)
                    del y2_tiles[c]
                p3_tiles[g] = p3

            def emit_exp(g, body):
                # Exp + bias-add for a finished group — placed at body end so
                # the epilogue engines drain this body's evictions first.
                gs = gsize(g)
                gp = gs * CP
                p3 = p3_tiles.pop(g)
                ex = exp_p.tile([GROUP * CP, NB], F16, tag="ex")
                nc.scalar.activation(ex[:gp, :], p3[:gp, :],
                                     AF.Exp, bias=b3t_sb[:gp, 0:1])
                y3 = y3p.tile([GROUP * CP, NB], F32, tag="y3")
                nc.vector.tensor_scalar(y3[:gp, :], p3[:gp, :],
                                        b3t_sb[:gp, 0:1], None, op0=ALU.add)
                ex_tiles[g] = ex
                y3_tiles[g] = y3
                pending_tails.append((g, body))

            def emit_softmax_tail(g):
                gs = gsize(g)
                gp = gs * CP
                psl = ps34.tile([GROUP * CP, NB], F32, tag="l34", name="psl")
                nc.tensor.matmul(psl[:gp, :], ones_sb[:gp, :gp],
                                 ex_tiles[g][:gp, :], start=True, stop=True)
                ls = lsp.tile([GROUP * CP, NB], F32, tag="ls")
                nc.scalar.activation(ls[:gp, :], psl[:gp, :], AF.Ln)
                o = op.tile([GROUP * CP, NB], F32, tag="o")
                nc.vector.tensor_tensor(o[:gp, :], y3_tiles[g][:gp, :],
                                        ls[:gp, :], op=ALU.subtract)
                ring = nc.scalar if gs < GROUP else nc.sync
                for m in range(gs):
                    cc = g * GROUP + m
                    ring.dma_start(out_d[:, cc * NB:(cc + 1) * NB],
                                   o[m * CP:m * CP + C, :])
                del ex_tiles[g], y3_tiles[g]

            def flush_tails(body):
                while pending_tails and pending_tails[0][1] < body:
                    emit_softmax_tail(pending_tails.pop(0)[0])

            def l1_mms(dst_psum, j, xs_t, xb_t, h):
                # Stripe part as one fp8 DoubleRow matmul (both K-planes in
                # a single pass). With only ~1/3 of the stream in DR mode,
                # the chip holds full clock (the all-DR variant throttled).
                nc.tensor.matmul(dst_psum, ws_sb[:, j, :, :],
                                 xs_t[:, h, :, :],
                                 start=True, stop=False, perf_mode=DR)
                nc.tensor.matmul(dst_psum, wb_sb[:, j, :],
                                 xb_t[:, h, j, :],
                                 start=False, stop=True)

            for c in range(n_chunk):
                s, h = c // (SC // NB), c % (SC // NB)
                xs_t, xb_t = xs_tiles[s], xb_tiles[s]

                # Deferred work from earlier chunks heads the PE stream:
                # their data deps resolved most of a body ago, so no stalls.
                if c >= 1:
                    emit_l2_all(c - 1)

                # Seven L1 blocks, each three fp8 matmuls into one PSUM
                # bank, evicted by a pure ReLU+cast alternating ACT/DVE.
                # y1 lands in DoubleRow pair tiles [OT, 2, NB] (blocks
                # 2k/2k+1 in the two K-planes); block 6 gets a flat tile
                # for the fp8 single tail matmul.
                y1s = []
                for j in range(n_blk):
                    p1 = ps1.tile([OT, NB], F32, tag="l1", name="p1")
                    l1_mms(p1[:], j, xs_t, xb_t, h)
                    if j < n_blk - 1:
                        if j % 2 == 0:
                            y1s.append(y1pp.tile([OT, 2, NB], F8, tag="y1", name="y1"))
                        dst = y1s[j // 2][:, j % 2, :]
                    else:
                        y1s.append(y1sp.tile([OT, NB], F8, tag="y1s", name="y1t"))
                        dst = y1s[3][:]
                    if j % 2 == 0 and j < n_blk - 1:
                        nc.scalar.activation(dst, p1[:], AF.Relu)
                    else:
                        nc.vector.tensor_scalar(dst, p1[:], 0.0, None,
                                                op0=ALU.max)
                y1_tiles[c] = y1s

                # Group tail work last: the epilogue engines finish this
                # body's evictions before touching exp/ln/subtract.
                if c >= 1 and ((c - 1) % GROUP == GROUP - 1
                               or c - 1 == n_chunk - 1):
                    g = (c - 1) // GROUP
                    emit_l3_group(g, c)
                    emit_exp(g, c)
                flush_tails(c)
                if c == n_chunk - 1:
                    # Start the final chunk's tail chain now: the PE waits
                    # briefly on this body's evictions, but that idle time
                    # would otherwise land in the drain anyway.
                    emit_l2_all(c)
                    g = c // GROUP
                    if c % GROUP != GROUP - 1:
                        emit_l3_group(g, c + 1)
                        emit_exp(g, c + 1)

            # ---- drain the pipeline ----
            for g, _ in list(pending_tails):
                emit_softmax_tail(g)
            pending_tails.clear()

    nc.compile()
    return nc


_CACHE = {}


def _prepare(x, W1, b1, W2, b2, W3, b3, mask1, mask2, mask3):
    B, D1 = x.shape
    H = W2.shape[0]
    C = W3.shape[0]
    assert B % N_CORES == 0
    Bc = B // N_CORES
    n_blk = D1 // OT
    n_pair = n_blk // 2
    n_sup = Bc // SC

    S, R_list = _decompose_mask1(np.asarray(mask1))
    nS = len(S)
    PS = (nS + 2 + 1) // 2              # stripe K-planes incl ones+zero rows
    maxR = max(len(r) for r in R_list)
    # >=65 partitions keeps the PE in its 128-row tile config: mixing 64-row
    # and 128-row matmuls in one stream costs a reconfig bubble per matmul.
    PB = max(maxR, 65)

    Wm1 = (np.asarray(W1) * np.asarray(mask1)).astype(np.float32)
    Wm2 = (np.asarray(W2) * np.asarray(mask2)).astype(np.float32)
    Wm3 = (np.asarray(W3) * np.asarray(mask3)).astype(np.float32)
    b1 = np.asarray(b1, np.float32)
    b2 = np.asarray(b2, np.float32)
    b3 = np.asarray(b3, np.float32)

    c8 = lambda a: np.asarray(a, dtype=NP8)
    c16 = lambda a: np.asarray(a, dtype=np.float16)

    # ---- stripe pack: K order = S cols, then ones row, then zero pad ----
    xT = np.asarray(x, np.float32).T                     # [D1, B]
    SP2 = 2 * PS
    xs_src = np.zeros((SP2, B), np.float32)
    xs_src[:nS] = xT[S]
    xs_src[nS] = 1.0
    # [NC, PS, n_sup, n_half, 2, NB] — chunk-major for contiguous slices
    n_half = SC // NB
    xs8 = (c8(xs_src).reshape(2, PS, N_CORES, n_sup, n_half, NB)
           .transpose(2, 1, 3, 4, 0, 5))
    xs8 = np.ascontiguousarray(xs8.reshape(N_CORES, PS, n_sup * 2 * SC))

    Ws_full = np.zeros((D1, SP2), np.float32)
    Ws_full[:, :nS] = Wm1[:, S]
    Ws_full[:, nS] = b1
    ws8 = np.ascontiguousarray(
        c8(Ws_full).reshape(n_blk, OT, 2, PS).transpose(3, 0, 2, 1)
        .reshape(PS, n_blk * 2 * OT))

    # ---- band pack (flat K, fp8 single matmuls) ----
    xb_src = np.zeros((n_blk, PB, B), np.float32)
    wb_src = np.zeros((n_blk, OT, PB), np.float32)
    for j, R in enumerate(R_list):
        xb_src[j, :len(R)] = xT[R]
        wb_src[j, :, :len(R)] = Wm1[j * OT:(j + 1) * OT, R]
    # [NC, PB, n_sup, n_half, n_blk, NB]
    xb8 = (c8(xb_src).reshape(n_blk, PB, N_CORES, n_sup, n_half, NB)
           .transpose(2, 1, 3, 4, 0, 5))
    xb8 = np.ascontiguousarray(
        xb8.reshape(N_CORES, PB, n_sup * n_blk * SC))
    wb8 = np.ascontiguousarray(
        c8(wb_src).transpose(2, 0, 1).reshape(PB, n_blk * OT))

    # ---- L2 pack: seven fp8 single matmuls ----
    t2 = Wm2.T.reshape(n_blk, OT, H)                     # [j, p, h]
    w2a8 = np.ascontiguousarray(
        c8(t2).transpose(1, 0, 2).reshape(OT, n_blk * H))

    # ---- L3 pack: classes padded to CP, 4 chunks per PE-tile group ----
    w3p = np.zeros((H, CP), np.float32)
    w3p[:, :C] = Wm3.T
    w3p16 = c16(w3p)
    GC = GROUP * CP
    ones = np.zeros((GC, GC), np.float32)
    for g in range(GROUP):
        ones[g * CP:g * CP + C, g * CP:g * CP + C] = 1.0
    ones16 = c16(ones)
    b3t = np.zeros((GC, 1), np.float32)
    for g in range(GROUP):
        b3t[g * CP:g * CP + C, 0] = b3
    b2p = b2.reshape(H, 1)

    meta = dict(PS=PS, PB=PB, Bc=Bc, D1=D1, H=H, C=C)
    key = (B, D1, H, C, nS, PB)
    if key not in _CACHE:
        _CACHE[key] = _build_program(meta)
    nc = _CACHE[key]

    in_maps = []
    for c in range(N_CORES):
        in_maps.append({
            "xs": xs8[c], "xb": xb8[c],
            "ws": ws8, "wb": wb8, "w2": w2a8,
            "w3": w3p16, "ones": ones16, "b2": b2p, "b3t": b3t,
        })
    return nc, in_maps, meta


def _assemble(results, meta):
    outs = [np.ascontiguousarray(results[c]["out"].T)     # [Bc, C]
            for c in range(N_CORES)]
    return np.concatenate(outs, axis=0).astype(np.float32)


def kernel(**inputs):
    nc, in_maps, meta = _prepare(**inputs)
    res = bass_utils.run_bass_kernel_spmd(nc, in_maps,
                                          core_ids=list(range(N_CORES)))
    return _assemble(res.results, meta)


def kernel_traced(tmpdir=None, **inputs):
    """Same as kernel() but with NTFF profiling; returns (output, results)."""
    nc, in_maps, meta = _prepare(**inputs)
    res = bass_utils.run_bass_kernel_spmd(nc, in_maps,
                                          core_ids=list(range(N_CORES)),
                                          trace=True, tmpdir=tmpdir)
    return _assemble(res.results, meta), res
